# revision 4
# baseline (speedup 1.0000x reference)
"""Sliding-window causal self-attention (GQA + RoPE + RMS-norm + value-embedding
gate) for Trainium2, sharded over 8 NeuronCores.

Sharding: sequence-parallel. (batch=2) x (4 sequence chunks of 1024) = 8 shards.
Each core computes attention for its own 1024 query rows. Window size = 1024 and
chunk size = 1024, so each core only needs K/V for its own chunk plus the
previous 1024 positions (halo). K/V (+rope/rms/gate) are recomputed locally for
the halo instead of communicated -> zero collectives. Chunk-0 shards get a
zero-padded halo; padded keys produce k=0 => exp(0)=1 which is corrected
exactly by subtracting the per-row pad count from the softmax denominator
(padded v rows are 0 so the numerator is untouched).

v2: fully software-pipelined single-pass emission. The kv projection (row-tile
rt), q projection (qt = rt-8), attention for qt, and the output projection for
qt-1 are interleaved in one loop so the PE instruction queue never starves
(phases B/C/D of v1 ran back-to-back; PE was only 61% busy). Other key points:
  - scores are computed pre-transposed (s[k,q] via lhsT=kT, rhs=qT) so the exp
    output (bf16) is directly the lhsT of the PV matmul; V is augmented with a
    ones column so PV emits y AND the softmax denominator in one accumulation.
  - one activation table for the whole kernel (exp/ln/copy): rsqrt is
    exp(-0.5*ln(ms+eps)), the sigmoid gate is 1/(1+exp(-u)) with the +1 and
    reciprocal on DVE. No 1.28us act-table reloads.
  - PSUM budget = exactly 8 banks: proj pool (kv/q/out, [512]f32 x2), score
    pieces [640]f32 + [512]f32, y [129]f32 x1, transpose pool x2.
  - engine balance: copies on Act, squares/reduces/normalize on DVE,
    k-rope + edge masks on Pool(gpsimd), exp on Act.
"""

import math
import sys

import numpy as np

sys.path.insert(0, "/opt/trn_rl_repo")

import ml_dtypes

import concourse.bass as bass
import concourse.bacc as bacc
import concourse.tile as tile
from concourse import mybir
from concourse import bass_utils

BF16 = ml_dtypes.bfloat16
F32 = np.float32

B, T, C = 2, 4096, 1024
H, HKV, D = 8, 2, 128
REP = H // HKV
WIN = 1024
RCHUNK = 1024          # own rows per core
E = 2048               # ext rows (halo + own)
NRT = E // 128         # 16 ext row tiles
NQT = RCHUNK // 128    # 8 q tiles
NKC = 9                # k chunks per q tile
NCA = 5                # chunks in score piece A
NCB = 4                # chunks in score piece B
NCT = C // 128         # 8 contraction tiles
EPS = float(np.finfo(np.float32).eps)
SCALE = 1.0 / math.sqrt(D)

dt = mybir.dt
AF = mybir.ActivationFunctionType
ALU = mybir.AluOpType
AX = mybir.AxisListType


def _bcast(ap, n, axis_pos=1):
    """Insert a 0-stride dim of size n into an AP at free-axis position."""
    new_ap = list(ap.ap)
    new_ap.insert(axis_pos, [0, n])
    return bass.AP(tensor=ap.tensor, offset=ap.offset, ap=new_ap)


def _halfswap(ap, nh):
    """View [128, nh, 128] with the two 64-wide halves of the last dim
    swapped: out[p, h, 0:64] = in[p, h, 64:128] and vice versa."""
    base = list(ap.ap)
    return bass.AP(tensor=ap.tensor, offset=ap.offset + 64,
                   ap=[base[0], base[1], [-64, 2], [1, 64]])


def build_nc():
    nc = bacc.Bacc("TRN2", target_bir_lowering=False, debug=False)

    xT_d = nc.dram_tensor("xT", [C, E], dt.bfloat16, kind="ExternalInput").ap()
    wq_d = nc.dram_tensor("wq", [C, C], dt.bfloat16, kind="ExternalInput").ap()
    wkv_d = nc.dram_tensor("wkv", [C, 512], dt.bfloat16, kind="ExternalInput").ap()
    wo_d = nc.dram_tensor("wo", [C, C], dt.bfloat16, kind="ExternalInput").ap()
    wg_d = nc.dram_tensor("wg", [32, HKV], dt.bfloat16, kind="ExternalInput").ap()
    ve_d = nc.dram_tensor("ve2", [E, HKV * D], dt.bfloat16, kind="ExternalInput").ap()
    cs_d = nc.dram_tensor("cs", [E, 256], dt.bfloat16, kind="ExternalInput").ap()
    tri_d = nc.dram_tensor("tri", [128, 2 * 128], dt.bfloat16, kind="ExternalInput").ap()
    npad_d = nc.dram_tensor("npad", [128, NQT], dt.float32, kind="ExternalInput").ap()
    id_d = nc.dram_tensor("ident", [128, 128], dt.bfloat16, kind="ExternalInput").ap()
    out_d = nc.dram_tensor("out", [RCHUNK, C], dt.float32, kind="ExternalOutput").ap()

    with tile.TileContext(nc) as tc:
        _body(tc, xT_d, wq_d, wkv_d, wo_d, wg_d, ve_d, cs_d, tri_d, npad_d, id_d,
              out_d)
    nc.compile()
    return nc


def _body(tc, xT_d, wq_d, wkv_d, wo_d, wg_d, ve_d, cs_d, tri_d, npad_d, id_d,
          out_d):
    nc = tc.nc
    from contextlib import ExitStack

    with ExitStack() as ctx:
        const = ctx.enter_context(tc.tile_pool(name="const", bufs=1))
        persist = ctx.enter_context(tc.tile_pool(name="persist", bufs=1))
        work = ctx.enter_context(tc.tile_pool(name="work", bufs=2))

        # ---- SBUF tensors ----
        wg_sb = const.tile([32, HKV], dt.bfloat16)
        id_sb = const.tile([128, 128], dt.bfloat16)
        tri_sb = const.tile([128, 2, 128], dt.bfloat16)
        npad_sb = const.tile([128, NQT], dt.float32)
        wkv_sb = const.tile([128, NCT, 512], dt.bfloat16)
        xT_sb = const.tile([128, NCT, E], dt.bfloat16)
        cs_sb = const.tile([128, NRT, 256], dt.bfloat16)
        ve_sb = const.tile([128, NRT, HKV * D], dt.bfloat16)
        wq_sb = const.tile([128, NCT, C], dt.bfloat16)
        wo_sb = const.tile([128, NCT, C], dt.bfloat16)
        eps_sb = const.tile([128, 1], dt.float32)

        kT_sb = persist.tile([128, HKV, NRT, 128], dt.bfloat16)   # [d, kvh, g, k]
        v_sb = persist.tile([128, NRT, HKV, 129], dt.bfloat16)    # [k, g, kvh, d|1]
        krot_sb = persist.tile([128, NRT, HKV * D], dt.bfloat16)  # roped k (pre-norm)
        msk_sb = persist.tile([128, NRT, HKV], dt.float32)
        gate_sb = persist.tile([128, NRT, HKV], dt.float32)

        nc.vector.memset(v_sb[:, :, :, 128:129], 1.0)
        nc.vector.memset(eps_sb, EPS)

        # ---- DMA: consumption order ----
        nc.sync.dma_start(out=wg_sb, in_=wg_d)
        nc.sync.dma_start(out=id_sb, in_=id_d)
        nc.sync.dma_start(out=tri_sb,
                          in_=tri_d.rearrange("p (a n) -> p a n", a=2))
        nc.sync.dma_start(out=npad_sb, in_=npad_d)
        nc.sync.dma_start(out=wkv_sb,
                          in_=wkv_d.rearrange("(a p) n -> p a n", p=128))
        xTv = xT_d.rearrange("(a p) n -> p a n", p=128)
        nc.sync.dma_start(out=xT_sb[:, 0, :], in_=xTv[:, 0, :])  # gate needs full ct0
        csv = cs_d.rearrange("(a p) n -> p a n", p=128)
        vev = ve_d.rearrange("(a p) n -> p a n", p=128)
        nc.sync.dma_start(out=cs_sb[:, 0:8, :], in_=csv[:, 0:8, :])
        nc.sync.dma_start(out=ve_sb[:, 0:8, :], in_=vev[:, 0:8, :])
        for ct in range(1, NCT):  # first halves: rows for rt 0..7
            nc.sync.dma_start(out=xT_sb[:, ct, 0:1024], in_=xTv[:, ct, 0:1024])

        # ---- gate for all rows: 1/(1+exp(-u)) (ve2 is pre-scaled by 2) ----
        with tc.tile_pool(name="gps", bufs=1, space="PSUM") as gps:
            g_psum = gps.tile([128, NRT * HKV], dt.float32)
            for rt in range(NRT):
                nc.tensor.matmul(g_psum[:, bass.ts(rt, HKV)],
                                 lhsT=xT_sb[0:32, 0, bass.ts(rt, 128)],
                                 rhs=wg_sb, start=True, stop=True)
            gflat = gate_sb.rearrange("p a n -> p (a n)")
            nc.scalar.activation(out=gflat, in_=g_psum, func=AF.Exp, scale=-1.0)
            nc.vector.tensor_scalar_add(gflat, gflat, 1.0)
            nc.vector.reciprocal(out=gflat, in_=gflat)

        # late DMAs (needed from rt=8 on)
        nc.sync.dma_start(out=cs_sb[:, 8:16, :], in_=csv[:, 8:16, :])
        nc.sync.dma_start(out=ve_sb[:, 8:16, :], in_=vev[:, 8:16, :])
        for ct in range(NCT):  # second halves: rows for rt 8..15
            nc.sync.dma_start(out=xT_sb[:, ct, 1024:2048], in_=xTv[:, ct, 1024:2048])
        nc.sync.dma_start(out=wq_sb,
                          in_=wq_d.rearrange("(a p) n -> p a n", p=128))
        nc.sync.dma_start(out=wo_sb,
                          in_=wo_d.rearrange("(a p) n -> p a n", p=128))

        # ---- main PSUM pools (8 banks total) ----
        projps = ctx.enter_context(tc.tile_pool(name="projps", bufs=2, space="PSUM"))
        sAps = ctx.enter_context(tc.tile_pool(name="sAps", bufs=1, space="PSUM"))
        sBps = ctx.enter_context(tc.tile_pool(name="sBps", bufs=1, space="PSUM"))
        yps = ctx.enter_context(tc.tile_pool(name="yps", bufs=1, space="PSUM"))
        tpps = ctx.enter_context(tc.tile_pool(name="tpps", bufs=2, space="PSUM"))

        # per-iteration ring state
        qT_cur = [None, None]      # qT tile ring (per qt)
        yN_cur = [None, None]      # yN tile ring (per qt)

        def kv_stage(rt):
            rs = bass.ts(rt, 128)
            kv = projps.tile([128, 512], dt.float32, tag="proj")
            for ct in range(NCT):
                nc.tensor.matmul(kv, lhsT=xT_sb[:, ct, rs], rhs=wkv_sb[:, ct, :],
                                 start=(ct == 0), stop=(ct == NCT - 1))
            # v = v_raw + gate*ve2  (DVE, reads psum)
            for kvh in range(HKV):
                nc.vector.scalar_tensor_tensor(
                    out=v_sb[:, rt, kvh, 0:128],
                    in0=ve_sb[:, rt, bass.ts(kvh, 128)],
                    scalar=gate_sb[:, rt, kvh:kvh + 1],
                    in1=kv[:, 256 + kvh * 128:256 + (kvh + 1) * 128],
                    op0=ALU.mult, op1=ALU.add)
            # k: psum -> sbuf copy on Act, rope on Pool
            kraw = work.tile([128, HKV * D], dt.bfloat16, tag="kraw")
            nc.scalar.copy(out=kraw, in_=kv[:, 0:256])
            k3 = kraw.rearrange("p (a n) -> p a n", a=HKV)
            kr3 = krot_sb[:, rt, :].rearrange("p (a n) -> p a n", a=HKV)
            ccb = _bcast(cs_sb[:, rt, 0:128], HKV)
            ssb = _bcast(cs_sb[:, rt, 128:256], HKV)
            kc_ = work.tile([128, HKV, 128], dt.bfloat16, tag="t1")
            ks_ = work.tile([128, HKV, 128], dt.bfloat16, tag="t2")
            nc.gpsimd.tensor_mul(kc_, k3, ccb)
            nc.gpsimd.tensor_mul(ks_, k3, ssb)
            nc.gpsimd.tensor_add(kr3, kc_, _halfswap(ks_, HKV))
            # rms stats from the ROPED values (exact for any cos/sin tables);
            # 1/sqrt via exp(-0.5*ln(ms+eps)) to stay in one act table
            ksq = work.tile([128, HKV, D], dt.bfloat16, tag="t1")
            nc.vector.tensor_mul(ksq, kr3, kr3)
            nc.vector.tensor_reduce(out=msk_sb[:, rt, :], in_=ksq,
                                    axis=AX.X, op=ALU.add)
            nc.scalar.activation(out=msk_sb[:, rt, :], in_=msk_sb[:, rt, :],
                                 func=AF.Ln, bias=eps_sb, scale=1.0 / D)
            nc.scalar.activation(out=msk_sb[:, rt, :], in_=msk_sb[:, rt, :],
                                 func=AF.Exp, scale=-0.5)
            for kvh in range(HKV):
                nc.vector.tensor_scalar_mul(
                    kr3[:, kvh, :], kr3[:, kvh, :], msk_sb[:, rt, kvh:kvh + 1])

        def ktr_stage(rt):
            kr3 = krot_sb[:, rt, :].rearrange("p (a n) -> p a n", a=HKV)
            for kvh in range(HKV):
                ktp = tpps.tile([128, 128], dt.bfloat16, tag="tp")
                nc.tensor.transpose(ktp, kr3[:, kvh, :], id_sb)
                nc.vector.tensor_copy(out=kT_sb[:, kvh, rt, :], in_=ktp)

        def q_stage(qt):
            rt = qt + NQT
            rs = bass.ts(rt, 128)
            qraw = work.tile([128, C], dt.bfloat16, tag="qraw")
            for half in range(2):
                qp = projps.tile([128, 512], dt.float32, tag="proj")
                for ct in range(NCT):
                    nc.tensor.matmul(qp, lhsT=xT_sb[:, ct, rs],
                                     rhs=wq_sb[:, ct, bass.ts(half, 512)],
                                     start=(ct == 0), stop=(ct == NCT - 1))
                nc.scalar.copy(out=qraw[:, bass.ts(half, 512)], in_=qp)
            qrot = work.tile([128, C], dt.bfloat16, tag="qrot")
            q3 = qraw.rearrange("p (a n) -> p a n", a=H)
            qr3 = qrot.rearrange("p (a n) -> p a n", a=H)
            ccbq = _bcast(cs_sb[:, rt, 0:128], H)
            ssbq = _bcast(cs_sb[:, rt, 128:256], H)
            u1 = work.tile([128, H, 128], dt.bfloat16, tag="u1")
            u2 = work.tile([128, H, 128], dt.bfloat16, tag="u2")
            nc.vector.tensor_mul(u1, q3, ccbq)
            nc.vector.tensor_mul(u2, q3, ssbq)
            nc.vector.tensor_add(qr3, u1, _halfswap(u2, H))
            qsq = work.tile([128, H, D], dt.bfloat16, tag="u1")
            msq = work.tile([128, H], dt.float32, tag="msq")
            nc.vector.tensor_mul(qsq, qr3, qr3)
            nc.vector.tensor_reduce(out=msq, in_=qsq, axis=AX.X, op=ALU.add)
            nc.scalar.activation(out=msq, in_=msq, func=AF.Ln,
                                 bias=eps_sb, scale=1.0 / D)
            nc.scalar.activation(out=msq, in_=msq, func=AF.Exp, scale=-0.5)
            for h in range(H):
                nc.vector.tensor_scalar(
                    out=qr3[:, h, :], in0=qr3[:, h, :],
                    scalar1=msq[:, h:h + 1], scalar2=SCALE,
                    op0=ALU.mult, op1=ALU.mult)
            return qrot

        def qtr_one(qrot, qT, h):
            qr3 = qrot.rearrange("p (a n) -> p a n", a=H)
            qtp = tpps.tile([128, 128], dt.bfloat16, tag="tp")
            nc.tensor.transpose(qtp, qr3[:, h, :], id_sb)
            nc.vector.tensor_copy(out=qT[:, h, :], in_=qtp)

        def d_slice(qt, yT, h):
            """One slice of the output projection for qt: transpose yN head h
            and accumulate its contribution into both output halves."""
            yN = yN_cur[qt % 2]
            ytp = tpps.tile([128, 128], dt.bfloat16, tag="tp")
            nc.tensor.transpose(ytp, yN[:, h, :], id_sb)
            nc.vector.tensor_copy(out=yT[:, h, :], in_=ytp)

        # output psum tiles for the in-flight D stage (allocated per qt)
        d_state = {}

        def d_open(qt):
            yT = work.tile([128, H, 128], dt.bfloat16, tag="yT")
            d_state[qt] = (yT, [None, None])

        def d_mm(qt, h):
            yT, oo = d_state[qt]
            if h == 0:
                oo[0] = projps.tile([128, 512], dt.float32, tag="proj", name="oo0")
                oo[1] = projps.tile([128, 512], dt.float32, tag="proj", name="oo1")
            for half in range(2):
                nc.tensor.matmul(oo[half], lhsT=yT[:, h, :],
                                 rhs=wo_sb[:, h, bass.ts(half, 512)],
                                 start=(h == 0), stop=(h == H - 1))

        def d_close(qt):
            yT, oo = d_state.pop(qt)
            for half in range(2):
                osb = work.tile([128, 512], dt.float32, tag="osb")
                nc.scalar.copy(out=osb, in_=oo[half])
                nc.sync.dma_start(
                    out=out_d[bass.ts(qt, 128), bass.ts(half, 512)], in_=osb)

        def att_step_scores(qt, h, pA, pB):
            kvh = h // REP
            sA = sAps.tile([128, NCA, 128], dt.float32, tag="sA")
            for kc in range(NCA):
                nc.tensor.matmul(sA[:, kc, :], lhsT=kT_sb[:, kvh, qt + kc, :],
                                 rhs=qT_cur[qt % 2][:, h, :],
                                 start=True, stop=True)
            sB = sBps.tile([128, NCB, 128], dt.float32, tag="sB")
            for kc in range(NCB):
                nc.tensor.matmul(sB[:, kc, :],
                                 lhsT=kT_sb[:, kvh, qt + NCA + kc, :],
                                 rhs=qT_cur[qt % 2][:, h, :],
                                 start=True, stop=True)
            nc.scalar.activation(out=pA.rearrange("p a n -> p (a n)"),
                                 in_=sA.rearrange("p a n -> p (a n)"),
                                 func=AF.Exp)
            nc.scalar.activation(out=pB.rearrange("p a n -> p (a n)"),
                                 in_=sB.rearrange("p a n -> p (a n)"),
                                 func=AF.Exp)
            nc.gpsimd.tensor_mul(pA[:, 0, :], pA[:, 0, :], tri_sb[:, 0, :])
            nc.gpsimd.tensor_mul(pB[:, NCB - 1, :], pB[:, NCB - 1, :],
                                 tri_sb[:, 1, :])

        def att_step_pv(qt, h, pA, pB):
            kvh = h // REP
            y = yps.tile([128, 129], dt.float32, tag="y")
            for kc in range(NKC):
                p = pA[:, kc, :] if kc < NCA else pB[:, kc - NCA, :]
                nc.tensor.matmul(y, lhsT=p, rhs=v_sb[:, qt + kc, kvh, :],
                                 start=(kc == 0), stop=(kc == NKC - 1))
            z = work.tile([128, 1], dt.float32, tag="z")
            nc.vector.tensor_sub(z, y[:, 128:129], npad_sb[:, qt:qt + 1])
            nc.vector.reciprocal(out=z, in_=z)
            nc.vector.tensor_scalar_mul(yN_cur[qt % 2][:, h, :], y[:, 0:128], z)

        # ================= warmup: halo kv (rt 0..7) =================
        for rt in range(NQT):
            kv_stage(rt)
            if rt > 0:
                ktr_stage(rt - 1)

        # ================= main loop (rt 8..15 / qt 0..7) =================
        pwork = ctx.enter_context(tc.tile_pool(name="pwork", bufs=2))
        for qt in range(NQT):
            rt = qt + NQT
            kv_stage(rt)
            qT_cur[qt % 2] = persist.tile([128, H, 128], dt.bfloat16,
                                          tag=f"qT{qt % 2}", name=f"qT_{qt}")
            yN_cur[qt % 2] = persist.tile([128, H, 128], dt.bfloat16,
                                          tag=f"yN{qt % 2}", name=f"yN_{qt}")
            qrot = q_stage(qt)
            if qt == 0:
                ktr_stage(rt - 1)  # ktr(7) from warmup lag
            ktr_stage(rt)
            if qt > 0:
                d_open(qt - 1)
            qtr_one(qrot, qT_cur[qt % 2], 0)
            # D head-start fillers to cover the q-chain latency
            if qt > 0:
                for j in range(4):
                    d_slice(qt - 1, d_state[qt - 1][0], j)
                    d_mm(qt - 1, j)

            pAs = [None, None]
            pBs = [None, None]
            for h in range(H):
                pAs[h % 2] = pwork.tile([128, NCA, 128], dt.bfloat16, tag="pA", name="pA")
                pBs[h % 2] = pwork.tile([128, NCB, 128], dt.bfloat16, tag="pB", name="pB")
                if h < H - 1:
                    qtr_one(qrot, qT_cur[qt % 2], h + 1)
                att_step_scores(qt, h, pAs[h % 2], pBs[h % 2])
                if h > 0:
                    att_step_pv(qt, h - 1, pAs[(h - 1) % 2], pBs[(h - 1) % 2])
                if qt > 0 and h < 4:
                    d_slice(qt - 1, d_state[qt - 1][0], 4 + h)
                    d_mm(qt - 1, 4 + h)
                if qt > 0 and h == 4:
                    d_close(qt - 1)
            att_step_pv(qt, H - 1, pAs[(H - 1) % 2], pBs[(H - 1) % 2])

        # final D for qt=7
        d_open(NQT - 1)
        for j in range(H):
            d_slice(NQT - 1, d_state[NQT - 1][0], j)
            d_mm(NQT - 1, j)
        d_close(NQT - 1)


# ---------------------------------------------------------------------------
# host side
# ---------------------------------------------------------------------------

def make_in_maps(x, ve, cos, sin, Wq, Wk, Wv, Wproj, Wg):
    """Build the 8 per-core input dicts (numpy, host-side prep)."""
    x = np.asarray(x, F32)
    ve = np.asarray(ve, F32)
    cos = np.asarray(cos, F32).reshape(T, 64)
    sin = np.asarray(sin, F32).reshape(T, 64)
    Wq = np.asarray(Wq, F32)
    Wk = np.asarray(Wk, F32)
    Wv = np.asarray(Wv, F32)
    Wproj = np.asarray(Wproj, F32)
    Wg = np.asarray(Wg, F32)

    wq = Wq.astype(BF16)
    wkv = np.concatenate([Wk, Wv], axis=1).astype(BF16)
    wo = Wproj.astype(BF16)
    wg = Wg.astype(BF16)
    ident = np.eye(128, dtype=BF16)

    # triangular masks in [k, q] layout
    kk = np.arange(128)[:, None]
    qq = np.arange(128)[None, :]
    tri = np.zeros((128, 2, 128), F32)
    tri[:, 0, :] = np.where(kk < qq, 0.0, 1.0)   # LEFT chunk (kc=0), mult mask
    tri[:, 1, :] = np.where(kk > qq, 0.0, 1.0)   # DIAG chunk (kc=8), mult mask
    tri = tri.reshape(128, 256).astype(BF16)

    in_maps = []
    for c in range(8):
        b, ck = divmod(c, 4)
        t0 = ck * RCHUNK
        es = t0 - WIN  # ext start (may be negative for chunk 0)
        pad = max(0, -es)

        def ext(a, fill_shape):
            out = np.zeros((E,) + fill_shape, F32)
            out[pad:] = a[es + pad: t0 + RCHUNK]
            return out

        x_e = ext(x[b], (C,))
        ve_e = ext(ve[b], (HKV * D,))
        cos_e = ext(cos, (64,))
        sin_e = ext(sin, (64,))

        npad = np.zeros((128, NQT), F32)
        if pad:
            kc = np.arange(NKC)[:, None]
            kl = np.arange(128)[None, :]
            r = np.arange(128)
            for qt in range(NQT):
                extpos = 128 * (qt + kc) + kl          # [9, 128]
                is_pad = extpos < pad
                for ri in r:
                    tri_ok = np.ones((NKC, 128), bool)
                    tri_ok[0] = kl[0] >= ri
                    tri_ok[NKC - 1] = kl[0] <= ri
                    npad[ri, qt] = np.sum(tri_ok & is_pad)

        in_maps.append({
            "xT": np.ascontiguousarray(x_e.T).astype(BF16),
            "wq": wq, "wkv": wkv, "wo": wo, "wg": wg,
            "ve2": (2.0 * ve_e).astype(BF16),
            "cs": np.concatenate([cos_e, cos_e, -sin_e, sin_e],
                                 axis=1).astype(BF16),
            "tri": tri, "npad": npad, "ident": ident,
        })
    return in_maps


_NC_CACHE = None


def kernel(x, ve, cos, sin, Wq, Wk, Wv, Wproj, Wg, window_size):
    assert int(window_size) == WIN
    global _NC_CACHE
    if _NC_CACHE is None:
        _NC_CACHE = build_nc()
    nc = _NC_CACHE
    in_maps = make_in_maps(x, ve, cos, sin, Wq, Wk, Wv, Wproj, Wg)
    res = bass_utils.run_bass_kernel_spmd(nc, in_maps, core_ids=list(range(8)))
    out = np.zeros((B, T, C), F32)
    for c in range(8):
        b, ck = divmod(c, 4)
        out[b, ck * RCHUNK:(ck + 1) * RCHUNK] = res.results[c]["out"]
    return out


# revision 5
# speedup vs baseline: 1.1077x; 1.1077x over previous
"""Sliding-window causal self-attention (GQA + RoPE + RMS-norm + value-embedding
gate) for Trainium2, sharded over 8 NeuronCores.

Sharding: sequence-parallel. (batch=2) x (4 sequence chunks of 1024) = 8 shards.
Each core computes attention for its own 1024 query rows. Window size = 1024 and
chunk size = 1024, so each core only needs K/V for its own chunk plus the
previous 1024 positions (halo). K/V (+rope/rms/gate) are recomputed locally for
the halo instead of communicated -> zero collectives. Chunk-0 shards get a
zero-padded halo; padded keys produce k=0 => exp(0)=1 which is corrected
exactly by subtracting the per-row pad count from the softmax denominator
(padded v rows are 0 so the numerator is untouched).

v2: fully software-pipelined single-pass emission. The kv projection (row-tile
rt), q projection (qt = rt-8), attention for qt, and the output projection for
qt-1 are interleaved in one loop so the PE instruction queue never starves
(phases B/C/D of v1 ran back-to-back; PE was only 61% busy). Other key points:
  - scores are computed pre-transposed (s[k,q] via lhsT=kT, rhs=qT) so the exp
    output (bf16) is directly the lhsT of the PV matmul; V is augmented with a
    ones column so PV emits y AND the softmax denominator in one accumulation.
  - one activation table for the whole kernel (exp/ln/copy): rsqrt is
    exp(-0.5*ln(ms+eps)), the sigmoid gate is 1/(1+exp(-u)) with the +1 and
    reciprocal on DVE. No 1.28us act-table reloads.
  - PSUM budget = exactly 8 banks: proj pool (kv/q/out, [512]f32 x2), score
    pieces [640]f32 + [512]f32, y [129]f32 x1, transpose pool x2.
  - engine balance: copies on Act, squares/reduces/normalize on DVE,
    k-rope + edge masks on Pool(gpsimd), exp on Act.
"""

import math
import sys

import numpy as np

sys.path.insert(0, "/opt/trn_rl_repo")

import ml_dtypes

import concourse.bass as bass
import concourse.bacc as bacc
import concourse.tile as tile
from concourse import mybir
from concourse import bass_utils

BF16 = ml_dtypes.bfloat16
F32 = np.float32

B, T, C = 2, 4096, 1024
H, HKV, D = 8, 2, 128
REP = H // HKV
WIN = 1024
RCHUNK = 1024          # own rows per core
E = 2048               # ext rows (halo + own)
NRT = E // 128         # 16 ext row tiles
NQT = RCHUNK // 128    # 8 q tiles
NKC = 9                # k chunks per q tile
NCA = 5                # chunks in score piece A
NCB = 4                # chunks in score piece B
NCT = C // 128         # 8 contraction tiles
EPS = float(np.finfo(np.float32).eps)
SCALE = 1.0 / math.sqrt(D)

dt = mybir.dt
AF = mybir.ActivationFunctionType
ALU = mybir.AluOpType
AX = mybir.AxisListType


def _bcast(ap, n, axis_pos=1):
    """Insert a 0-stride dim of size n into an AP at free-axis position."""
    new_ap = list(ap.ap)
    new_ap.insert(axis_pos, [0, n])
    return bass.AP(tensor=ap.tensor, offset=ap.offset, ap=new_ap)


def _halfswap(ap, nh):
    """View [128, nh, 128] with the two 64-wide halves of the last dim
    swapped: out[p, h, 0:64] = in[p, h, 64:128] and vice versa."""
    base = list(ap.ap)
    return bass.AP(tensor=ap.tensor, offset=ap.offset + 64,
                   ap=[base[0], base[1], [-64, 2], [1, 64]])


class _Bacc(bacc.Bacc):
    """Bacc whose act-table chooser is steered to the one table that holds
    every activation function this kernel uses (exp, ln, copy), so the whole
    program needs a single 1.28us table load instead of thrashing between
    exp_and_others and natural_log on every rms-norm. Table IDs stay
    canonical (indices into the real act_info.json list); only the guidance
    sets passed to the greedy chooser are filtered."""

    def insert_act_table_loads(self):
        import bass_rust as _br
        from concourse.hw_specs import get_activation_tables

        has_act = any(isinstance(i, mybir.InstActivation)
                      for b in self.main_func.blocks
                      for i in b.instructions)
        if not has_act:
            return
        tables = list(get_activation_tables(self.m.arch).items())
        want = {AF.Exp, AF.Ln, AF.Copy}
        best = next((i for i, (_, fs) in enumerate(tables) if want <= fs), None)
        if best is not None:
            tables = [(nm, fs if i == best else fs - want)
                      for i, (nm, fs) in enumerate(tables)]
        _br.insert_act_table_loads(self, tables)


def build_nc():
    nc = _Bacc("TRN2", target_bir_lowering=False, debug=False)

    xT_d = nc.dram_tensor("xT", [C, E], dt.bfloat16, kind="ExternalInput").ap()
    wq_d = nc.dram_tensor("wq", [C, C], dt.bfloat16, kind="ExternalInput").ap()
    wkv_d = nc.dram_tensor("wkv", [C, 512], dt.bfloat16, kind="ExternalInput").ap()
    wo_d = nc.dram_tensor("wo", [C, C], dt.bfloat16, kind="ExternalInput").ap()
    wg_d = nc.dram_tensor("wg", [32, HKV], dt.bfloat16, kind="ExternalInput").ap()
    ve_d = nc.dram_tensor("ve2", [E, HKV * D], dt.bfloat16, kind="ExternalInput").ap()
    cs_d = nc.dram_tensor("cs", [E, 256], dt.bfloat16, kind="ExternalInput").ap()
    tri_d = nc.dram_tensor("tri", [128, 2 * 128], dt.bfloat16, kind="ExternalInput").ap()
    npad_d = nc.dram_tensor("npad", [128, NQT], dt.float32, kind="ExternalInput").ap()
    id_d = nc.dram_tensor("ident", [128, 128], dt.bfloat16, kind="ExternalInput").ap()
    out_d = nc.dram_tensor("out", [RCHUNK, C], dt.float32, kind="ExternalOutput").ap()

    with tile.TileContext(nc) as tc:
        _body(tc, xT_d, wq_d, wkv_d, wo_d, wg_d, ve_d, cs_d, tri_d, npad_d, id_d,
              out_d)
    nc.compile()
    return nc


def _body(tc, xT_d, wq_d, wkv_d, wo_d, wg_d, ve_d, cs_d, tri_d, npad_d, id_d,
          out_d):
    nc = tc.nc
    from contextlib import ExitStack

    with ExitStack() as ctx:
        const = ctx.enter_context(tc.tile_pool(name="const", bufs=1))
        persist = ctx.enter_context(tc.tile_pool(name="persist", bufs=1))
        work = ctx.enter_context(tc.tile_pool(name="work", bufs=2))

        # ---- SBUF tensors ----
        wg_sb = const.tile([32, HKV], dt.bfloat16)
        id_sb = const.tile([128, 128], dt.bfloat16)
        tri_sb = const.tile([128, 2, 128], dt.bfloat16)
        npad_sb = const.tile([128, NQT], dt.float32)
        wkv_sb = const.tile([128, NCT, 512], dt.bfloat16)
        xT_sb = const.tile([128, NCT, E], dt.bfloat16)
        cs_sb = const.tile([128, NRT, 256], dt.bfloat16)
        ve_sb = const.tile([128, NRT, HKV * D], dt.bfloat16)
        wq_sb = const.tile([128, NCT, C], dt.bfloat16)
        wo_sb = const.tile([128, NCT, C], dt.bfloat16)
        eps_sb = const.tile([128, 1], dt.float32)

        kT_sb = persist.tile([128, HKV, NRT, 128], dt.bfloat16)   # [d, kvh, g, k]
        v_sb = persist.tile([128, NRT, HKV, 129], dt.bfloat16)    # [k, g, kvh, d|1]
        krot_sb = persist.tile([128, NRT, HKV * D], dt.bfloat16)  # roped k (pre-norm)
        msk_sb = persist.tile([128, NRT, HKV], dt.float32)
        gate_sb = persist.tile([128, NRT, HKV], dt.float32)

        nc.vector.memset(v_sb[:, :, :, 128:129], 1.0)
        nc.vector.memset(eps_sb, EPS)

        # ---- DMA: consumption order ----
        nc.sync.dma_start(out=wg_sb, in_=wg_d)
        nc.sync.dma_start(out=id_sb, in_=id_d)
        nc.sync.dma_start(out=tri_sb,
                          in_=tri_d.rearrange("p (a n) -> p a n", a=2))
        nc.sync.dma_start(out=npad_sb, in_=npad_d)
        nc.sync.dma_start(out=wkv_sb,
                          in_=wkv_d.rearrange("(a p) n -> p a n", p=128))
        xTv = xT_d.rearrange("(a p) n -> p a n", p=128)
        nc.sync.dma_start(out=xT_sb[:, 0, :], in_=xTv[:, 0, :])  # gate needs full ct0
        csv = cs_d.rearrange("(a p) n -> p a n", p=128)
        vev = ve_d.rearrange("(a p) n -> p a n", p=128)
        nc.sync.dma_start(out=cs_sb[:, 0:8, :], in_=csv[:, 0:8, :])
        nc.sync.dma_start(out=ve_sb[:, 0:8, :], in_=vev[:, 0:8, :])
        for ct in range(1, NCT):  # first halves: rows for rt 0..7
            nc.sync.dma_start(out=xT_sb[:, ct, 0:1024], in_=xTv[:, ct, 0:1024])

        # ---- gate for all rows: 1/(1+exp(-u)) (ve2 is pre-scaled by 2) ----
        with tc.tile_pool(name="gps", bufs=1, space="PSUM") as gps:
            g_psum = gps.tile([128, NRT * HKV], dt.float32)
            for rt in range(NRT):
                nc.tensor.matmul(g_psum[:, bass.ts(rt, HKV)],
                                 lhsT=xT_sb[0:32, 0, bass.ts(rt, 128)],
                                 rhs=wg_sb, start=True, stop=True)
            gflat = gate_sb.rearrange("p a n -> p (a n)")
            nc.scalar.activation(out=gflat, in_=g_psum, func=AF.Exp, scale=-1.0)
            nc.vector.tensor_scalar_add(gflat, gflat, 1.0)
            nc.vector.reciprocal(out=gflat, in_=gflat)

        # late DMAs (needed from rt=8 on)
        nc.sync.dma_start(out=cs_sb[:, 8:16, :], in_=csv[:, 8:16, :])
        nc.sync.dma_start(out=ve_sb[:, 8:16, :], in_=vev[:, 8:16, :])
        for ct in range(NCT):  # second halves: rows for rt 8..15
            nc.sync.dma_start(out=xT_sb[:, ct, 1024:2048], in_=xTv[:, ct, 1024:2048])
        nc.sync.dma_start(out=wq_sb,
                          in_=wq_d.rearrange("(a p) n -> p a n", p=128))
        nc.sync.dma_start(out=wo_sb,
                          in_=wo_d.rearrange("(a p) n -> p a n", p=128))

        # ---- main PSUM pools (8 banks total) ----
        projps = ctx.enter_context(tc.tile_pool(name="projps", bufs=2, space="PSUM"))
        sAps = ctx.enter_context(tc.tile_pool(name="sAps", bufs=1, space="PSUM"))
        sBps = ctx.enter_context(tc.tile_pool(name="sBps", bufs=1, space="PSUM"))
        yps = ctx.enter_context(tc.tile_pool(name="yps", bufs=1, space="PSUM"))
        tpps = ctx.enter_context(tc.tile_pool(name="tpps", bufs=2, space="PSUM"))

        # per-iteration ring state
        qT_cur = [None, None]      # qT tile ring (per qt)
        yN_cur = [None, None]      # yN tile ring (per qt)

        def kv_stage(rt):
            rs = bass.ts(rt, 128)
            kv = projps.tile([128, 512], dt.float32, tag="proj")
            for ct in range(NCT):
                nc.tensor.matmul(kv, lhsT=xT_sb[:, ct, rs], rhs=wkv_sb[:, ct, :],
                                 start=(ct == 0), stop=(ct == NCT - 1))
            # v = v_raw + gate*ve2  (DVE, reads psum)
            for kvh in range(HKV):
                nc.vector.scalar_tensor_tensor(
                    out=v_sb[:, rt, kvh, 0:128],
                    in0=ve_sb[:, rt, bass.ts(kvh, 128)],
                    scalar=gate_sb[:, rt, kvh:kvh + 1],
                    in1=kv[:, 256 + kvh * 128:256 + (kvh + 1) * 128],
                    op0=ALU.mult, op1=ALU.add)
            # k: psum -> sbuf copy on Act, rope on Pool
            kraw = work.tile([128, HKV * D], dt.bfloat16, tag="kraw")
            nc.scalar.copy(out=kraw, in_=kv[:, 0:256])
            k3 = kraw.rearrange("p (a n) -> p a n", a=HKV)
            kr3 = krot_sb[:, rt, :].rearrange("p (a n) -> p a n", a=HKV)
            ccb = _bcast(cs_sb[:, rt, 0:128], HKV)
            ssb = _bcast(cs_sb[:, rt, 128:256], HKV)
            kc_ = work.tile([128, HKV, 128], dt.bfloat16, tag="t1")
            ks_ = work.tile([128, HKV, 128], dt.bfloat16, tag="t2")
            nc.gpsimd.tensor_mul(kc_, k3, ccb)
            nc.gpsimd.tensor_mul(ks_, k3, ssb)
            nc.gpsimd.tensor_add(kr3, kc_, _halfswap(ks_, HKV))
            # rms stats from the ROPED values (exact for any cos/sin tables);
            # 1/sqrt via exp(-0.5*ln(ms+eps)) to stay in one act table
            ksq = work.tile([128, HKV, D], dt.bfloat16, tag="t1")
            nc.vector.tensor_mul(ksq, kr3, kr3)
            nc.vector.tensor_reduce(out=msk_sb[:, rt, :], in_=ksq,
                                    axis=AX.X, op=ALU.add)
            nc.scalar.activation(out=msk_sb[:, rt, :], in_=msk_sb[:, rt, :],
                                 func=AF.Ln, bias=eps_sb, scale=1.0 / D)
            nc.scalar.activation(out=msk_sb[:, rt, :], in_=msk_sb[:, rt, :],
                                 func=AF.Exp, scale=-0.5)
            for kvh in range(HKV):
                nc.vector.tensor_scalar_mul(
                    kr3[:, kvh, :], kr3[:, kvh, :], msk_sb[:, rt, kvh:kvh + 1])

        def ktr_stage(rt):
            kr3 = krot_sb[:, rt, :].rearrange("p (a n) -> p a n", a=HKV)
            for kvh in range(HKV):
                ktp = tpps.tile([128, 128], dt.bfloat16, tag="tp")
                nc.tensor.transpose(ktp, kr3[:, kvh, :], id_sb)
                nc.vector.tensor_copy(out=kT_sb[:, kvh, rt, :], in_=ktp)

        def q_stage(qt):
            rt = qt + NQT
            rs = bass.ts(rt, 128)
            qraw = work.tile([128, C], dt.bfloat16, tag="qraw")
            for half in range(2):
                qp = projps.tile([128, 512], dt.float32, tag="proj")
                for ct in range(NCT):
                    nc.tensor.matmul(qp, lhsT=xT_sb[:, ct, rs],
                                     rhs=wq_sb[:, ct, bass.ts(half, 512)],
                                     start=(ct == 0), stop=(ct == NCT - 1))
                nc.scalar.copy(out=qraw[:, bass.ts(half, 512)], in_=qp)
            qrot = work.tile([128, C], dt.bfloat16, tag="qrot")
            q3 = qraw.rearrange("p (a n) -> p a n", a=H)
            qr3 = qrot.rearrange("p (a n) -> p a n", a=H)
            ccbq = _bcast(cs_sb[:, rt, 0:128], H)
            ssbq = _bcast(cs_sb[:, rt, 128:256], H)
            u1 = work.tile([128, H, 128], dt.bfloat16, tag="u1")
            u2 = work.tile([128, H, 128], dt.bfloat16, tag="u2")
            nc.vector.tensor_mul(u1, q3, ccbq)
            nc.vector.tensor_mul(u2, q3, ssbq)
            nc.vector.tensor_add(qr3, u1, _halfswap(u2, H))
            qsq = work.tile([128, H, D], dt.bfloat16, tag="u1")
            msq = work.tile([128, H], dt.float32, tag="msq")
            nc.vector.tensor_mul(qsq, qr3, qr3)
            nc.vector.tensor_reduce(out=msq, in_=qsq, axis=AX.X, op=ALU.add)
            nc.scalar.activation(out=msq, in_=msq, func=AF.Ln,
                                 bias=eps_sb, scale=1.0 / D)
            nc.scalar.activation(out=msq, in_=msq, func=AF.Exp, scale=-0.5)
            for h in range(H):
                nc.vector.tensor_scalar(
                    out=qr3[:, h, :], in0=qr3[:, h, :],
                    scalar1=msq[:, h:h + 1], scalar2=SCALE,
                    op0=ALU.mult, op1=ALU.mult)
            return qrot

        def qtr_one(qrot, qT, h):
            qr3 = qrot.rearrange("p (a n) -> p a n", a=H)
            qtp = tpps.tile([128, 128], dt.bfloat16, tag="tp")
            nc.tensor.transpose(qtp, qr3[:, h, :], id_sb)
            nc.vector.tensor_copy(out=qT[:, h, :], in_=qtp)

        def d_slice(qt, yT, h):
            """One slice of the output projection for qt: transpose yN head h
            and accumulate its contribution into both output halves."""
            yN = yN_cur[qt % 2]
            ytp = tpps.tile([128, 128], dt.bfloat16, tag="tp")
            nc.tensor.transpose(ytp, yN[:, h, :], id_sb)
            nc.vector.tensor_copy(out=yT[:, h, :], in_=ytp)

        # output psum tiles for the in-flight D stage (allocated per qt)
        d_state = {}

        def d_open(qt):
            yT = work.tile([128, H, 128], dt.bfloat16, tag="yT")
            d_state[qt] = (yT, [None, None])

        def d_mm(qt, h):
            yT, oo = d_state[qt]
            if h == 0:
                oo[0] = projps.tile([128, 512], dt.float32, tag="proj", name="oo0")
                oo[1] = projps.tile([128, 512], dt.float32, tag="proj", name="oo1")
            for half in range(2):
                nc.tensor.matmul(oo[half], lhsT=yT[:, h, :],
                                 rhs=wo_sb[:, h, bass.ts(half, 512)],
                                 start=(h == 0), stop=(h == H - 1))

        def d_close(qt):
            yT, oo = d_state.pop(qt)
            for half in range(2):
                osb = work.tile([128, 512], dt.float32, tag="osb")
                nc.scalar.copy(out=osb, in_=oo[half])
                nc.sync.dma_start(
                    out=out_d[bass.ts(qt, 128), bass.ts(half, 512)], in_=osb)

        def att_step_scores(qt, h, pA, pB):
            kvh = h // REP
            sA = sAps.tile([128, NCA, 128], dt.float32, tag="sA")
            for kc in range(NCA):
                nc.tensor.matmul(sA[:, kc, :], lhsT=kT_sb[:, kvh, qt + kc, :],
                                 rhs=qT_cur[qt % 2][:, h, :],
                                 start=True, stop=True)
            sB = sBps.tile([128, NCB, 128], dt.float32, tag="sB")
            for kc in range(NCB):
                nc.tensor.matmul(sB[:, kc, :],
                                 lhsT=kT_sb[:, kvh, qt + NCA + kc, :],
                                 rhs=qT_cur[qt % 2][:, h, :],
                                 start=True, stop=True)
            nc.scalar.activation(out=pA.rearrange("p a n -> p (a n)"),
                                 in_=sA.rearrange("p a n -> p (a n)"),
                                 func=AF.Exp)
            nc.scalar.activation(out=pB.rearrange("p a n -> p (a n)"),
                                 in_=sB.rearrange("p a n -> p (a n)"),
                                 func=AF.Exp)
            nc.gpsimd.tensor_mul(pA[:, 0, :], pA[:, 0, :], tri_sb[:, 0, :])
            nc.gpsimd.tensor_mul(pB[:, NCB - 1, :], pB[:, NCB - 1, :],
                                 tri_sb[:, 1, :])

        def att_step_pv(qt, h, pA, pB):
            kvh = h // REP
            y = yps.tile([128, 129], dt.float32, tag="y")
            for kc in range(NKC):
                p = pA[:, kc, :] if kc < NCA else pB[:, kc - NCA, :]
                nc.tensor.matmul(y, lhsT=p, rhs=v_sb[:, qt + kc, kvh, :],
                                 start=(kc == 0), stop=(kc == NKC - 1))
            z = work.tile([128, 1], dt.float32, tag="z")
            nc.vector.tensor_sub(z, y[:, 128:129], npad_sb[:, qt:qt + 1])
            nc.vector.reciprocal(out=z, in_=z)
            nc.vector.tensor_scalar_mul(yN_cur[qt % 2][:, h, :], y[:, 0:128], z)

        # ================= warmup: halo kv (rt 0..7) =================
        for rt in range(NQT):
            kv_stage(rt)
            if rt > 0:
                ktr_stage(rt - 1)

        # ================= main loop (rt 8..15 / qt 0..7) =================
        pwork = ctx.enter_context(tc.tile_pool(name="pwork", bufs=2))
        for qt in range(NQT):
            rt = qt + NQT
            kv_stage(rt)
            qT_cur[qt % 2] = persist.tile([128, H, 128], dt.bfloat16,
                                          tag=f"qT{qt % 2}", name=f"qT_{qt}")
            yN_cur[qt % 2] = persist.tile([128, H, 128], dt.bfloat16,
                                          tag=f"yN{qt % 2}", name=f"yN_{qt}")
            qrot = q_stage(qt)
            if qt == 0:
                ktr_stage(rt - 1)  # ktr(7) from warmup lag
            ktr_stage(rt)
            if qt > 0:
                d_open(qt - 1)
            qtr_one(qrot, qT_cur[qt % 2], 0)
            # D head-start fillers to cover the q-chain latency
            if qt > 0:
                for j in range(4):
                    d_slice(qt - 1, d_state[qt - 1][0], j)
                    d_mm(qt - 1, j)

            pAs = [None, None]
            pBs = [None, None]
            for h in range(H):
                pAs[h % 2] = pwork.tile([128, NCA, 128], dt.bfloat16, tag="pA", name="pA")
                pBs[h % 2] = pwork.tile([128, NCB, 128], dt.bfloat16, tag="pB", name="pB")
                if h < H - 1:
                    qtr_one(qrot, qT_cur[qt % 2], h + 1)
                att_step_scores(qt, h, pAs[h % 2], pBs[h % 2])
                if h > 0:
                    att_step_pv(qt, h - 1, pAs[(h - 1) % 2], pBs[(h - 1) % 2])
                if qt > 0 and h < 4:
                    d_slice(qt - 1, d_state[qt - 1][0], 4 + h)
                    d_mm(qt - 1, 4 + h)
                if qt > 0 and h == 4:
                    d_close(qt - 1)
            att_step_pv(qt, H - 1, pAs[(H - 1) % 2], pBs[(H - 1) % 2])

        # final D for qt=7
        d_open(NQT - 1)
        for j in range(H):
            d_slice(NQT - 1, d_state[NQT - 1][0], j)
            d_mm(NQT - 1, j)
        d_close(NQT - 1)


# ---------------------------------------------------------------------------
# host side
# ---------------------------------------------------------------------------

def make_in_maps(x, ve, cos, sin, Wq, Wk, Wv, Wproj, Wg):
    """Build the 8 per-core input dicts (numpy, host-side prep)."""
    x = np.asarray(x, F32)
    ve = np.asarray(ve, F32)
    cos = np.asarray(cos, F32).reshape(T, 64)
    sin = np.asarray(sin, F32).reshape(T, 64)
    Wq = np.asarray(Wq, F32)
    Wk = np.asarray(Wk, F32)
    Wv = np.asarray(Wv, F32)
    Wproj = np.asarray(Wproj, F32)
    Wg = np.asarray(Wg, F32)

    wq = Wq.astype(BF16)
    wkv = np.concatenate([Wk, Wv], axis=1).astype(BF16)
    wo = Wproj.astype(BF16)
    wg = Wg.astype(BF16)
    ident = np.eye(128, dtype=BF16)

    # triangular masks in [k, q] layout
    kk = np.arange(128)[:, None]
    qq = np.arange(128)[None, :]
    tri = np.zeros((128, 2, 128), F32)
    tri[:, 0, :] = np.where(kk < qq, 0.0, 1.0)   # LEFT chunk (kc=0), mult mask
    tri[:, 1, :] = np.where(kk > qq, 0.0, 1.0)   # DIAG chunk (kc=8), mult mask
    tri = tri.reshape(128, 256).astype(BF16)

    in_maps = []
    for c in range(8):
        b, ck = divmod(c, 4)
        t0 = ck * RCHUNK
        es = t0 - WIN  # ext start (may be negative for chunk 0)
        pad = max(0, -es)

        def ext(a, fill_shape):
            out = np.zeros((E,) + fill_shape, F32)
            out[pad:] = a[es + pad: t0 + RCHUNK]
            return out

        x_e = ext(x[b], (C,))
        ve_e = ext(ve[b], (HKV * D,))
        cos_e = ext(cos, (64,))
        sin_e = ext(sin, (64,))

        npad = np.zeros((128, NQT), F32)
        if pad:
            kc = np.arange(NKC)[:, None]
            kl = np.arange(128)[None, :]
            r = np.arange(128)
            for qt in range(NQT):
                extpos = 128 * (qt + kc) + kl          # [9, 128]
                is_pad = extpos < pad
                for ri in r:
                    tri_ok = np.ones((NKC, 128), bool)
                    tri_ok[0] = kl[0] >= ri
                    tri_ok[NKC - 1] = kl[0] <= ri
                    npad[ri, qt] = np.sum(tri_ok & is_pad)

        in_maps.append({
            "xT": np.ascontiguousarray(x_e.T).astype(BF16),
            "wq": wq, "wkv": wkv, "wo": wo, "wg": wg,
            "ve2": (2.0 * ve_e).astype(BF16),
            "cs": np.concatenate([cos_e, cos_e, -sin_e, sin_e],
                                 axis=1).astype(BF16),
            "tri": tri, "npad": npad, "ident": ident,
        })
    return in_maps


_NC_CACHE = None


def kernel(x, ve, cos, sin, Wq, Wk, Wv, Wproj, Wg, window_size):
    assert int(window_size) == WIN
    global _NC_CACHE
    if _NC_CACHE is None:
        _NC_CACHE = build_nc()
    nc = _NC_CACHE
    in_maps = make_in_maps(x, ve, cos, sin, Wq, Wk, Wv, Wproj, Wg)
    res = bass_utils.run_bass_kernel_spmd(nc, in_maps, core_ids=list(range(8)))
    out = np.zeros((B, T, C), F32)
    for c in range(8):
        b, ck = divmod(c, 4)
        out[b, ck * RCHUNK:(ck + 1) * RCHUNK] = res.results[c]["out"]
    return out


# revision 9
# speedup vs baseline: 1.3178x; 1.1897x over previous
"""Sliding-window causal self-attention (GQA + RoPE + RMS-norm + value-embedding
gate) for Trainium2, sharded over 8 NeuronCores.

Sharding: sequence-parallel. (batch=2) x (4 sequence chunks of 1024) = 8 shards.
Each core computes attention for its own 1024 query rows. Window size = 1024 and
chunk size = 1024, so each core only needs K/V for its own chunk plus the
previous 1024 positions (halo). K/V (+rope/rms/gate) are recomputed locally for
the halo instead of communicated -> zero collectives. Chunk-0 shards get a
zero-padded halo; padded keys produce k=0 => exp(0)=1 which is corrected
exactly by subtracting the per-row pad count from the softmax denominator
(padded v rows are 0 so the numerator is untouched).

v2: fully software-pipelined single-pass emission. The kv projection (row-tile
rt), q projection (qt = rt-8), attention for qt, and the output projection for
qt-1 are interleaved in one loop so the PE instruction queue never starves
(phases B/C/D of v1 ran back-to-back; PE was only 61% busy). Other key points:
  - scores are computed pre-transposed (s[k,q] via lhsT=kT, rhs=qT) so the exp
    output (bf16) is directly the lhsT of the PV matmul; V is augmented with a
    ones column so PV emits y AND the softmax denominator in one accumulation.
  - one activation table for the whole kernel (exp/ln/copy): rsqrt is
    exp(-0.5*ln(ms+eps)), the sigmoid gate is 1/(1+exp(-u)) with the +1 and
    reciprocal on DVE. No 1.28us act-table reloads.
  - PSUM budget = exactly 8 banks: proj pool (kv/q/out, [512]f32 x2), score
    pieces [640]f32 + [512]f32, y [129]f32 x1, transpose pool x2.
  - engine balance: copies on Act, squares/reduces/normalize on DVE,
    k-rope + edge masks on Pool(gpsimd), exp on Act.
"""

import math
import sys

import numpy as np

sys.path.insert(0, "/opt/trn_rl_repo")

import ml_dtypes

import concourse.bass as bass
import concourse.bacc as bacc
import concourse.tile as tile
from concourse import mybir
from concourse import bass_utils

BF16 = ml_dtypes.bfloat16
F32 = np.float32

B, T, C = 2, 4096, 1024
H, HKV, D = 8, 2, 128
REP = H // HKV
WIN = 1024
RCHUNK = 1024          # own rows per core
E = 2048               # ext rows (halo + own)
NRT = E // 128         # 16 ext row tiles
NQT = RCHUNK // 128    # 8 q tiles
NKC = 9                # k chunks per q tile
NCA = 5                # chunks in score piece A
NCB = 4                # chunks in score piece B
NCT = C // 128         # 8 contraction tiles
EPS = float(np.finfo(np.float32).eps)
SCALE = 1.0 / math.sqrt(D)

dt = mybir.dt
AF = mybir.ActivationFunctionType
ALU = mybir.AluOpType
AX = mybir.AxisListType


def _bcast(ap, n, axis_pos=1):
    """Insert a 0-stride dim of size n into an AP at free-axis position."""
    new_ap = list(ap.ap)
    new_ap.insert(axis_pos, [0, n])
    return bass.AP(tensor=ap.tensor, offset=ap.offset, ap=new_ap)


def _halfswap(ap, nh):
    """View [128, nh, 128] with the two 64-wide halves of the last dim
    swapped: out[p, h, 0:64] = in[p, h, 64:128] and vice versa."""
    base = list(ap.ap)
    return bass.AP(tensor=ap.tensor, offset=ap.offset + 64,
                   ap=[base[0], base[1], [-64, 2], [1, 64]])


class _Bacc(bacc.Bacc):
    """Bacc whose act-table chooser is steered to the one table that holds
    every activation function this kernel uses (exp, ln, copy), so the whole
    program needs a single 1.28us table load instead of thrashing between
    exp_and_others and natural_log on every rms-norm. Table IDs stay
    canonical (indices into the real act_info.json list); only the guidance
    sets passed to the greedy chooser are filtered."""

    def insert_act_table_loads(self):
        import bass_rust as _br
        from concourse.hw_specs import get_activation_tables

        has_act = any(isinstance(i, mybir.InstActivation)
                      for b in self.main_func.blocks
                      for i in b.instructions)
        if not has_act:
            return
        tables = list(get_activation_tables(self.m.arch).items())
        want = {AF.Exp, AF.Ln, AF.Copy}
        best = next((i for i, (_, fs) in enumerate(tables) if want <= fs), None)
        if best is not None:
            tables = [(nm, fs if i == best else fs - want)
                      for i, (nm, fs) in enumerate(tables)]
        _br.insert_act_table_loads(self, tables)


def build_nc():
    nc = _Bacc("TRN2", target_bir_lowering=False, debug=False)

    xT_d = nc.dram_tensor("xT", [C, E], dt.bfloat16, kind="ExternalInput").ap()
    wq_d = nc.dram_tensor("wq", [C, C], dt.bfloat16, kind="ExternalInput").ap()
    wkv_d = nc.dram_tensor("wkv", [C, 512], dt.bfloat16, kind="ExternalInput").ap()
    wo_d = nc.dram_tensor("wo", [C, C], dt.bfloat16, kind="ExternalInput").ap()
    wg_d = nc.dram_tensor("wg", [32, HKV], dt.bfloat16, kind="ExternalInput").ap()
    ve_d = nc.dram_tensor("ve2", [E, HKV * D], dt.bfloat16, kind="ExternalInput").ap()
    cs_d = nc.dram_tensor("cs", [E, 256], dt.bfloat16, kind="ExternalInput").ap()
    tri_d = nc.dram_tensor("tri", [128, 2 * 128], dt.bfloat16, kind="ExternalInput").ap()
    npad_d = nc.dram_tensor("npad", [128, NQT], dt.float32, kind="ExternalInput").ap()
    id_d = nc.dram_tensor("ident", [128, 128], dt.bfloat16, kind="ExternalInput").ap()
    out_d = nc.dram_tensor("out", [RCHUNK, C], dt.float32, kind="ExternalOutput").ap()

    with tile.TileContext(nc) as tc:
        _body(tc, xT_d, wq_d, wkv_d, wo_d, wg_d, ve_d, cs_d, tri_d, npad_d, id_d,
              out_d)
    nc.compile()
    return nc


def _body(tc, xT_d, wq_d, wkv_d, wo_d, wg_d, ve_d, cs_d, tri_d, npad_d, id_d,
          out_d):
    nc = tc.nc
    from contextlib import ExitStack

    with ExitStack() as ctx:
        const = ctx.enter_context(tc.tile_pool(name="const", bufs=1))
        persist = ctx.enter_context(tc.tile_pool(name="persist", bufs=1))
        work = ctx.enter_context(tc.tile_pool(name="work", bufs=2))

        # ---- SBUF tensors ----
        wg_sb = const.tile([32, HKV], dt.bfloat16)
        id_sb = const.tile([128, 128], dt.bfloat16)
        tri_sb = const.tile([128, 2, 128], dt.bfloat16)
        npad_sb = const.tile([128, NQT], dt.float32)
        wkv_sb = const.tile([128, NCT, 512], dt.bfloat16)
        xT_sb = const.tile([128, NCT, E], dt.bfloat16)
        cs_sb = const.tile([128, NRT, 256], dt.bfloat16)
        ve_sb = const.tile([128, NRT, HKV * D], dt.bfloat16)
        wq_sb = const.tile([128, NCT, C], dt.bfloat16)
        wo_sb = const.tile([128, NCT, C], dt.bfloat16)
        eps_sb = const.tile([128, 1], dt.float32)

        kT_sb = persist.tile([128, HKV, NRT, 128], dt.bfloat16)   # [d, kvh, g, k]
        v_sb = persist.tile([128, NRT, HKV, 129], dt.bfloat16)    # [k, g, kvh, d|1]
        krot_sb = persist.tile([128, NRT, HKV * D], dt.bfloat16)  # roped k (pre-norm)
        msk_sb = persist.tile([128, NRT, HKV], dt.float32)
        gate_sb = persist.tile([128, NRT, HKV], dt.float32)

        nc.vector.memset(v_sb[:, :, :, 128:129], 1.0)
        nc.vector.memset(eps_sb, EPS)

        # ---- DMA: consumption order ----
        nc.sync.dma_start(out=wg_sb, in_=wg_d)
        nc.sync.dma_start(out=id_sb, in_=id_d)
        nc.sync.dma_start(out=tri_sb,
                          in_=tri_d.rearrange("p (a n) -> p a n", a=2))
        nc.sync.dma_start(out=npad_sb, in_=npad_d)
        nc.sync.dma_start(out=wkv_sb,
                          in_=wkv_d.rearrange("(a p) n -> p a n", p=128))
        xTv = xT_d.rearrange("(a p) n -> p a n", p=128)
        nc.sync.dma_start(out=xT_sb[:, 0, :], in_=xTv[:, 0, :])  # gate needs full ct0
        csv = cs_d.rearrange("(a p) n -> p a n", p=128)
        vev = ve_d.rearrange("(a p) n -> p a n", p=128)
        nc.sync.dma_start(out=cs_sb[:, 0:8, :], in_=csv[:, 0:8, :])
        nc.sync.dma_start(out=ve_sb[:, 0:8, :], in_=vev[:, 0:8, :])
        for ct in range(1, NCT):  # first halves: rows for rt 0..7
            nc.sync.dma_start(out=xT_sb[:, ct, 0:1024], in_=xTv[:, ct, 0:1024])

        # ---- gate for all rows: 1/(1+exp(-u)) (ve2 is pre-scaled by 2) ----
        with tc.tile_pool(name="gps", bufs=1, space="PSUM") as gps:
            g_psum = gps.tile([128, NRT * HKV], dt.float32)
            for rt in range(NRT):
                nc.tensor.matmul(g_psum[:, bass.ts(rt, HKV)],
                                 lhsT=xT_sb[0:32, 0, bass.ts(rt, 128)],
                                 rhs=wg_sb, start=True, stop=True)
            gflat = gate_sb.rearrange("p a n -> p (a n)")
            nc.scalar.activation(out=gflat, in_=g_psum, func=AF.Exp, scale=-1.0)
            nc.vector.tensor_scalar_add(gflat, gflat, 1.0)
            nc.vector.reciprocal(out=gflat, in_=gflat)

        # late DMAs (needed from rt=8 on)
        nc.sync.dma_start(out=cs_sb[:, 8:16, :], in_=csv[:, 8:16, :])
        nc.sync.dma_start(out=ve_sb[:, 8:16, :], in_=vev[:, 8:16, :])
        for ct in range(NCT):  # second halves: rows for rt 8..15
            nc.sync.dma_start(out=xT_sb[:, ct, 1024:2048], in_=xTv[:, ct, 1024:2048])
        nc.sync.dma_start(out=wq_sb,
                          in_=wq_d.rearrange("(a p) n -> p a n", p=128))
        nc.sync.dma_start(out=wo_sb,
                          in_=wo_d.rearrange("(a p) n -> p a n", p=128))

        # ---- main PSUM pools (8 banks total) ----
        projps = ctx.enter_context(tc.tile_pool(name="projps", bufs=2, space="PSUM"))
        sAps = ctx.enter_context(tc.tile_pool(name="sAps", bufs=1, space="PSUM"))
        sBps = ctx.enter_context(tc.tile_pool(name="sBps", bufs=1, space="PSUM"))
        yps = ctx.enter_context(tc.tile_pool(name="yps", bufs=1, space="PSUM"))
        tpps = ctx.enter_context(tc.tile_pool(name="tpps", bufs=2, space="PSUM"))

        # per-iteration ring state
        qT_cur = [None, None]      # qT tile ring (per qt)
        yN_cur = [None, None]      # yN tile ring (per qt)

        def kv_stage(rt):
            rs = bass.ts(rt, 128)
            kv = projps.tile([128, 512], dt.float32, tag="proj")
            for ct in range(NCT):
                nc.tensor.matmul(kv, lhsT=xT_sb[:, ct, rs], rhs=wkv_sb[:, ct, :],
                                 start=(ct == 0), stop=(ct == NCT - 1))
            # v = v_raw + gate*ve2  (DVE, reads psum)
            for kvh in range(HKV):
                nc.vector.scalar_tensor_tensor(
                    out=v_sb[:, rt, kvh, 0:128],
                    in0=ve_sb[:, rt, bass.ts(kvh, 128)],
                    scalar=gate_sb[:, rt, kvh:kvh + 1],
                    in1=kv[:, 256 + kvh * 128:256 + (kvh + 1) * 128],
                    op0=ALU.mult, op1=ALU.add)
            # k: psum -> sbuf copy on Act, rope on Pool
            kraw = work.tile([128, HKV * D], dt.bfloat16, tag="kraw")
            nc.scalar.copy(out=kraw, in_=kv[:, 0:256])
            k3 = kraw.rearrange("p (a n) -> p a n", a=HKV)
            kr3 = krot_sb[:, rt, :].rearrange("p (a n) -> p a n", a=HKV)
            ccb = _bcast(cs_sb[:, rt, 0:128], HKV)
            ssb = _bcast(cs_sb[:, rt, 128:256], HKV)
            kc_ = work.tile([128, HKV, 128], dt.bfloat16, tag="t1")
            ks_ = work.tile([128, HKV, 128], dt.bfloat16, tag="t2")
            nc.gpsimd.tensor_mul(kc_, k3, ccb)
            nc.gpsimd.tensor_mul(ks_, k3, ssb)
            nc.gpsimd.tensor_add(kr3, kc_, _halfswap(ks_, HKV))
            # rms stats from the ROPED values (exact for any cos/sin tables);
            # 1/sqrt via exp(-0.5*ln(ms+eps)) to stay in one act table
            ksq = work.tile([128, HKV, D], dt.bfloat16, tag="t1")
            nc.vector.tensor_mul(ksq, kr3, kr3)
            nc.vector.tensor_reduce(out=msk_sb[:, rt, :], in_=ksq,
                                    axis=AX.X, op=ALU.add)
            nc.scalar.activation(out=msk_sb[:, rt, :], in_=msk_sb[:, rt, :],
                                 func=AF.Ln, bias=eps_sb, scale=1.0 / D)
            nc.scalar.activation(out=msk_sb[:, rt, :], in_=msk_sb[:, rt, :],
                                 func=AF.Exp, scale=-0.5)
            for kvh in range(HKV):
                nc.vector.tensor_scalar_mul(
                    kr3[:, kvh, :], kr3[:, kvh, :], msk_sb[:, rt, kvh:kvh + 1])

        def ktr_stage(rt):
            kr3 = krot_sb[:, rt, :].rearrange("p (a n) -> p a n", a=HKV)
            for kvh in range(HKV):
                ktp = tpps.tile([128, 128], dt.bfloat16, tag="tp")
                nc.tensor.transpose(ktp, kr3[:, kvh, :], id_sb)
                nc.vector.tensor_copy(out=kT_sb[:, kvh, rt, :], in_=ktp)

        def q_half(qt, half, qraw, qrot, msq):
            """Project + rope + rms-normalize heads [4*half, 4*half+4) so the
            first heads' qT is ready long before the second half is needed."""
            rt = qt + NQT
            rs = bass.ts(rt, 128)
            HH = H // 2
            qp = projps.tile([128, 512], dt.float32, tag="proj", name="qp")
            for ct in range(NCT):
                nc.tensor.matmul(qp, lhsT=xT_sb[:, ct, rs],
                                 rhs=wq_sb[:, ct, bass.ts(half, 512)],
                                 start=(ct == 0), stop=(ct == NCT - 1))
            nc.scalar.copy(out=qraw[:, bass.ts(half, 512)], in_=qp)
            q3 = qraw[:, bass.ts(half, 512)].rearrange("p (a n) -> p a n", a=HH)
            qr3 = qrot[:, bass.ts(half, 512)].rearrange("p (a n) -> p a n", a=HH)
            ccbq = _bcast(cs_sb[:, rt, 0:128], HH)
            ssbq = _bcast(cs_sb[:, rt, 128:256], HH)
            u1 = work.tile([128, HH, 128], dt.bfloat16, tag="u1")
            u2 = work.tile([128, HH, 128], dt.bfloat16, tag="u2")
            nc.vector.tensor_mul(u1, q3, ccbq)
            nc.vector.tensor_mul(u2, q3, ssbq)
            nc.vector.tensor_add(qr3, u1, _halfswap(u2, HH))
            qsq = work.tile([128, HH, D], dt.bfloat16, tag="u1")
            nc.vector.tensor_mul(qsq, qr3, qr3)
            ms = msq[:, bass.ts(half, HH)]
            nc.vector.tensor_reduce(out=ms, in_=qsq, axis=AX.X, op=ALU.add)
            nc.scalar.activation(out=ms, in_=ms, func=AF.Ln,
                                 bias=eps_sb, scale=1.0 / D)
            nc.scalar.activation(out=ms, in_=ms, func=AF.Exp, scale=-0.5)
            for h in range(HH):
                nc.vector.tensor_scalar(
                    out=qr3[:, h, :], in0=qr3[:, h, :],
                    scalar1=ms[:, h:h + 1], scalar2=SCALE,
                    op0=ALU.mult, op1=ALU.mult)

        def qtr_one(qrot, qT, h):
            qr3 = qrot.rearrange("p (a n) -> p a n", a=H)
            qtp = tpps.tile([128, 128], dt.bfloat16, tag="tp")
            nc.tensor.transpose(qtp, qr3[:, h, :], id_sb)
            nc.vector.tensor_copy(out=qT[:, h, :], in_=qtp)

        def d_tr(qt, yT, h):
            """Transpose yN head h of qt into yT (PE + DVE copy)."""
            yN = yN_cur[qt % 2]
            ytp = tpps.tile([128, 128], dt.bfloat16, tag="tp")
            nc.tensor.transpose(ytp, yN[:, h, :], id_sb)
            nc.vector.tensor_copy(out=yT[:, h, :], in_=ytp)

        # output psum tiles for the in-flight D stage (allocated per qt)
        d_state = {}

        def d_open(qt):
            yT = work.tile([128, H, 128], dt.bfloat16, tag="yT")
            d_state[qt] = (yT, [None, None])

        def d_mm(qt, h):
            yT, oo = d_state[qt]
            if h == 0:
                oo[0] = projps.tile([128, 512], dt.float32, tag="proj", name="oo0")
                oo[1] = projps.tile([128, 512], dt.float32, tag="proj", name="oo1")
            for half in range(2):
                nc.tensor.matmul(oo[half], lhsT=yT[:, h, :],
                                 rhs=wo_sb[:, h, bass.ts(half, 512)],
                                 start=(h == 0), stop=(h == H - 1))

        def d_close(qt):
            yT, oo = d_state.pop(qt)
            for half in range(2):
                osb = work.tile([128, 512], dt.float32, tag="osb")
                nc.scalar.copy(out=osb, in_=oo[half])
                nc.sync.dma_start(
                    out=out_d[bass.ts(qt, 128), bass.ts(half, 512)], in_=osb)

        def att_step_scores(qt, h, pA, pB):
            kvh = h // REP
            sA = sAps.tile([128, NCA, 128], dt.float32, tag="sA")
            for kc in range(NCA):
                nc.tensor.matmul(sA[:, kc, :], lhsT=kT_sb[:, kvh, qt + kc, :],
                                 rhs=qT_cur[qt % 2][:, h, :],
                                 start=True, stop=True)
            sB = sBps.tile([128, NCB, 128], dt.float32, tag="sB")
            for kc in range(NCB):
                nc.tensor.matmul(sB[:, kc, :],
                                 lhsT=kT_sb[:, kvh, qt + NCA + kc, :],
                                 rhs=qT_cur[qt % 2][:, h, :],
                                 start=True, stop=True)
            nc.scalar.activation(out=pA.rearrange("p a n -> p (a n)"),
                                 in_=sA.rearrange("p a n -> p (a n)"),
                                 func=AF.Exp)
            nc.scalar.activation(out=pB.rearrange("p a n -> p (a n)"),
                                 in_=sB.rearrange("p a n -> p (a n)"),
                                 func=AF.Exp)
            nc.gpsimd.tensor_mul(pA[:, 0, :], pA[:, 0, :], tri_sb[:, 0, :])
            nc.gpsimd.tensor_mul(pB[:, NCB - 1, :], pB[:, NCB - 1, :],
                                 tri_sb[:, 1, :])

        def att_step_pv(qt, h, pA, pB):
            kvh = h // REP
            y = yps.tile([128, 129], dt.float32, tag="y")
            for kc in range(NKC):
                p = pA[:, kc, :] if kc < NCA else pB[:, kc - NCA, :]
                nc.tensor.matmul(y, lhsT=p, rhs=v_sb[:, qt + kc, kvh, :],
                                 start=(kc == 0), stop=(kc == NKC - 1))
            z = work.tile([128, 1], dt.float32, tag="z")
            nc.vector.tensor_sub(z, y[:, 128:129], npad_sb[:, qt:qt + 1])
            nc.vector.reciprocal(out=z, in_=z)
            nc.vector.tensor_scalar_mul(yN_cur[qt % 2][:, h, :], y[:, 0:128], z)

        # ================= warmup: halo kv (rt 0..7) =================
        for rt in range(NQT):
            kv_stage(rt)
            if rt > 0:
                ktr_stage(rt - 1)

        # ================= main loop (rt 8..15 / qt 0..7) =================
        pwork = ctx.enter_context(tc.tile_pool(name="pwork", bufs=2))
        for qt in range(NQT):
            rt = qt + NQT
            kv_stage(rt)
            qT_cur[qt % 2] = persist.tile([128, H, 128], dt.bfloat16,
                                          tag=f"qT{qt % 2}", name=f"qT_{qt}")
            yN_cur[qt % 2] = persist.tile([128, H, 128], dt.bfloat16,
                                          tag=f"yN{qt % 2}", name=f"yN_{qt}")
            qraw = work.tile([128, C], dt.bfloat16, tag="qraw")
            qrot = work.tile([128, C], dt.bfloat16, tag="qrot")
            msq = work.tile([128, H], dt.float32, tag="msq")
            q_half(qt, 0, qraw, qrot, msq)
            # D fillers for qt-1: PE work that depends on nothing current,
            # keeping PE busy while the k- and q-chains (Act/DVE/Pool) run.
            if qt > 0:
                d_open(qt - 1)
                yT = d_state[qt - 1][0]
                d_tr(qt - 1, yT, 0)
                d_tr(qt - 1, yT, 1)
                for j in range(6):
                    d_mm(qt - 1, j)
                    if j + 2 < 6:
                        d_tr(qt - 1, yT, j + 2)
            q_half(qt, 1, qraw, qrot, msq)
            if qt == 0:
                ktr_stage(rt - 1)  # ktr(7) from warmup lag
            ktr_stage(rt)
            qtr_one(qrot, qT_cur[qt % 2], 0)

            pAs = [None, None]
            pBs = [None, None]
            for h in range(H):
                pAs[h % 2] = pwork.tile([128, NCA, 128], dt.bfloat16, tag="pA", name="pA")
                pBs[h % 2] = pwork.tile([128, NCB, 128], dt.bfloat16, tag="pB", name="pB")
                if h < H - 1:
                    qtr_one(qrot, qT_cur[qt % 2], h + 1)
                att_step_scores(qt, h, pAs[h % 2], pBs[h % 2])
                if h > 0:
                    att_step_pv(qt, h - 1, pAs[(h - 1) % 2], pBs[(h - 1) % 2])
                if qt > 0 and h < 2:
                    d_tr(qt - 1, d_state[qt - 1][0], 6 + h)
                    d_mm(qt - 1, 6 + h)
                if qt > 0 and h == 2:
                    d_close(qt - 1)
            att_step_pv(qt, H - 1, pAs[(H - 1) % 2], pBs[(H - 1) % 2])

        # final D for qt=7
        d_open(NQT - 1)
        yT = d_state[NQT - 1][0]
        d_tr(NQT - 1, yT, 0)
        d_tr(NQT - 1, yT, 1)
        for j in range(H):
            d_mm(NQT - 1, j)
            if j + 2 < H:
                d_tr(NQT - 1, yT, j + 2)
        d_close(NQT - 1)


# ---------------------------------------------------------------------------
# host side
# ---------------------------------------------------------------------------

def make_in_maps(x, ve, cos, sin, Wq, Wk, Wv, Wproj, Wg):
    """Build the 8 per-core input dicts (numpy, host-side prep)."""
    x = np.asarray(x, F32)
    ve = np.asarray(ve, F32)
    cos = np.asarray(cos, F32).reshape(T, 64)
    sin = np.asarray(sin, F32).reshape(T, 64)
    Wq = np.asarray(Wq, F32)
    Wk = np.asarray(Wk, F32)
    Wv = np.asarray(Wv, F32)
    Wproj = np.asarray(Wproj, F32)
    Wg = np.asarray(Wg, F32)

    wq = Wq.astype(BF16)
    wkv = np.concatenate([Wk, Wv], axis=1).astype(BF16)
    wo = Wproj.astype(BF16)
    wg = Wg.astype(BF16)
    ident = np.eye(128, dtype=BF16)

    # triangular masks in [k, q] layout
    kk = np.arange(128)[:, None]
    qq = np.arange(128)[None, :]
    tri = np.zeros((128, 2, 128), F32)
    tri[:, 0, :] = np.where(kk < qq, 0.0, 1.0)   # LEFT chunk (kc=0), mult mask
    tri[:, 1, :] = np.where(kk > qq, 0.0, 1.0)   # DIAG chunk (kc=8), mult mask
    tri = tri.reshape(128, 256).astype(BF16)

    in_maps = []
    for c in range(8):
        b, ck = divmod(c, 4)
        t0 = ck * RCHUNK
        es = t0 - WIN  # ext start (may be negative for chunk 0)
        pad = max(0, -es)

        def ext(a, fill_shape):
            out = np.zeros((E,) + fill_shape, F32)
            out[pad:] = a[es + pad: t0 + RCHUNK]
            return out

        x_e = ext(x[b], (C,))
        ve_e = ext(ve[b], (HKV * D,))
        cos_e = ext(cos, (64,))
        sin_e = ext(sin, (64,))

        npad = np.zeros((128, NQT), F32)
        if pad:
            kc = np.arange(NKC)[:, None]
            kl = np.arange(128)[None, :]
            r = np.arange(128)
            for qt in range(NQT):
                extpos = 128 * (qt + kc) + kl          # [9, 128]
                is_pad = extpos < pad
                for ri in r:
                    tri_ok = np.ones((NKC, 128), bool)
                    tri_ok[0] = kl[0] >= ri
                    tri_ok[NKC - 1] = kl[0] <= ri
                    npad[ri, qt] = np.sum(tri_ok & is_pad)

        in_maps.append({
            "xT": np.ascontiguousarray(x_e.T).astype(BF16),
            "wq": wq, "wkv": wkv, "wo": wo, "wg": wg,
            "ve2": (2.0 * ve_e).astype(BF16),
            "cs": np.concatenate([cos_e, cos_e, -sin_e, sin_e],
                                 axis=1).astype(BF16),
            "tri": tri, "npad": npad, "ident": ident,
        })
    return in_maps


_NC_CACHE = None


def kernel(x, ve, cos, sin, Wq, Wk, Wv, Wproj, Wg, window_size):
    assert int(window_size) == WIN
    global _NC_CACHE
    if _NC_CACHE is None:
        _NC_CACHE = build_nc()
    nc = _NC_CACHE
    in_maps = make_in_maps(x, ve, cos, sin, Wq, Wk, Wv, Wproj, Wg)
    res = bass_utils.run_bass_kernel_spmd(nc, in_maps, core_ids=list(range(8)))
    out = np.zeros((B, T, C), F32)
    for c in range(8):
        b, ck = divmod(c, 4)
        out[b, ck * RCHUNK:(ck + 1) * RCHUNK] = res.results[c]["out"]
    return out


# revision 13
# speedup vs baseline: 1.3743x; 1.0429x over previous
"""Sliding-window causal self-attention (GQA + RoPE + RMS-norm + value-embedding
gate) for Trainium2, sharded over 8 NeuronCores.

Sharding: sequence-parallel. (batch=2) x (4 sequence chunks of 1024) = 8 shards.
Each core computes attention for its own 1024 query rows. Window size = 1024 and
chunk size = 1024, so each core only needs K/V for its own chunk plus the
previous 1024 positions (halo). K/V (+rope/rms/gate) are recomputed locally for
the halo instead of communicated -> zero collectives. Chunk-0 shards get a
zero-padded halo; padded keys produce k=0 => exp(0)=1 which is corrected
exactly by subtracting the per-row pad count from the softmax denominator
(padded v rows are 0 so the numerator is untouched).

v2: fully software-pipelined single-pass emission. The kv projection (row-tile
rt), q projection (qt = rt-8), attention for qt, and the output projection for
qt-1 are interleaved in one loop so the PE instruction queue never starves
(phases B/C/D of v1 ran back-to-back; PE was only 61% busy). Other key points:
  - scores are computed pre-transposed (s[k,q] via lhsT=kT, rhs=qT) so the exp
    output (bf16) is directly the lhsT of the PV matmul; V is augmented with a
    ones column so PV emits y AND the softmax denominator in one accumulation.
  - one activation table for the whole kernel (exp/ln/copy): rsqrt is
    exp(-0.5*ln(ms+eps)), the sigmoid gate is 1/(1+exp(-u)) with the +1 and
    reciprocal on DVE. No 1.28us act-table reloads.
  - PSUM budget = exactly 8 banks: proj pool (kv/q/out, [512]f32 x2), score
    pieces [640]f32 + [512]f32, y [129]f32 x1, transpose pool x2.
  - engine balance: copies on Act, squares/reduces/normalize on DVE,
    k-rope + edge masks on Pool(gpsimd), exp on Act.
"""

import math
import sys

import numpy as np

sys.path.insert(0, "/opt/trn_rl_repo")

import ml_dtypes

import concourse.bass as bass
import concourse.bacc as bacc
import concourse.tile as tile
from concourse import mybir
from concourse import bass_utils

BF16 = ml_dtypes.bfloat16
F32 = np.float32

B, T, C = 2, 4096, 1024
H, HKV, D = 8, 2, 128
REP = H // HKV
WIN = 1024
RCHUNK = 1024          # own rows per core
E = 2048               # ext rows (halo + own)
NRT = E // 128         # 16 ext row tiles
NQT = RCHUNK // 128    # 8 q tiles
NKC = 9                # k chunks per q tile
NCA = 5                # chunks in score piece A
NCB = 4                # chunks in score piece B
NCT = C // 128         # 8 contraction tiles
EPS = float(np.finfo(np.float32).eps)
SCALE = 1.0 / math.sqrt(D)

dt = mybir.dt
AF = mybir.ActivationFunctionType
ALU = mybir.AluOpType
AX = mybir.AxisListType


def _bcast(ap, n, axis_pos=1):
    """Insert a 0-stride dim of size n into an AP at free-axis position."""
    new_ap = list(ap.ap)
    new_ap.insert(axis_pos, [0, n])
    return bass.AP(tensor=ap.tensor, offset=ap.offset, ap=new_ap)


def _halfswap(ap, nh):
    """View [128, nh, 128] with the two 64-wide halves of the last dim
    swapped: out[p, h, 0:64] = in[p, h, 64:128] and vice versa."""
    base = list(ap.ap)
    return bass.AP(tensor=ap.tensor, offset=ap.offset + 64,
                   ap=[base[0], base[1], [-64, 2], [1, 64]])


class _Bacc(bacc.Bacc):
    """Bacc whose act-table chooser is steered to the one table that holds
    every activation function this kernel uses (exp, ln, copy), so the whole
    program needs a single 1.28us table load instead of thrashing between
    exp_and_others and natural_log on every rms-norm. Table IDs stay
    canonical (indices into the real act_info.json list); only the guidance
    sets passed to the greedy chooser are filtered."""

    def insert_act_table_loads(self):
        import bass_rust as _br
        from concourse.hw_specs import get_activation_tables

        has_act = any(isinstance(i, mybir.InstActivation)
                      for b in self.main_func.blocks
                      for i in b.instructions)
        if not has_act:
            return
        tables = list(get_activation_tables(self.m.arch).items())
        want = {AF.Exp, AF.Ln, AF.Copy}
        best = next((i for i, (_, fs) in enumerate(tables) if want <= fs), None)
        if best is not None:
            tables = [(nm, fs if i == best else fs - want)
                      for i, (nm, fs) in enumerate(tables)]
        _br.insert_act_table_loads(self, tables)


def build_nc():
    nc = _Bacc("TRN2", target_bir_lowering=False, debug=False)

    xT_d = nc.dram_tensor("xT", [C, E], dt.bfloat16, kind="ExternalInput").ap()
    wq_d = nc.dram_tensor("wq", [C, C], dt.bfloat16, kind="ExternalInput").ap()
    wkv_d = nc.dram_tensor("wkv", [C, 512], dt.bfloat16, kind="ExternalInput").ap()
    wo_d = nc.dram_tensor("wo", [C, C], dt.bfloat16, kind="ExternalInput").ap()
    wg_d = nc.dram_tensor("wg", [32, HKV], dt.bfloat16, kind="ExternalInput").ap()
    ve_d = nc.dram_tensor("ve2", [E, HKV * D], dt.bfloat16, kind="ExternalInput").ap()
    cs_d = nc.dram_tensor("cs", [E, 256], dt.bfloat16, kind="ExternalInput").ap()
    tri_d = nc.dram_tensor("tri", [128, 2 * 128], dt.bfloat16, kind="ExternalInput").ap()
    npad_d = nc.dram_tensor("npad", [128, NQT], dt.float32, kind="ExternalInput").ap()
    id_d = nc.dram_tensor("ident", [128, 128], dt.bfloat16, kind="ExternalInput").ap()
    out_d = nc.dram_tensor("out", [RCHUNK, C], dt.float32, kind="ExternalOutput").ap()

    with tile.TileContext(nc) as tc:
        _body(tc, xT_d, wq_d, wkv_d, wo_d, wg_d, ve_d, cs_d, tri_d, npad_d, id_d,
              out_d)
    nc.compile()
    return nc


def _body(tc, xT_d, wq_d, wkv_d, wo_d, wg_d, ve_d, cs_d, tri_d, npad_d, id_d,
          out_d):
    nc = tc.nc
    from contextlib import ExitStack

    with ExitStack() as ctx:
        const = ctx.enter_context(tc.tile_pool(name="const", bufs=1))
        persist = ctx.enter_context(tc.tile_pool(name="persist", bufs=1))
        work = ctx.enter_context(tc.tile_pool(name="work", bufs=2))

        # ---- SBUF tensors ----
        wg_sb = const.tile([32, HKV], dt.bfloat16)
        id_sb = const.tile([128, 128], dt.bfloat16)
        tri_sb = const.tile([128, 2, 128], dt.bfloat16)
        npad_sb = const.tile([128, NQT], dt.float32)
        wkv_sb = const.tile([128, NCT, 512], dt.bfloat16)
        xT_sb = const.tile([128, NCT, E], dt.bfloat16)
        cs_sb = const.tile([128, NRT, 256], dt.bfloat16)
        ve_sb = const.tile([128, NRT, HKV * D], dt.bfloat16)
        wq_sb = const.tile([128, NCT, C], dt.bfloat16)
        wo_sb = const.tile([128, NCT, C], dt.bfloat16)
        eps_sb = const.tile([128, 1], dt.float32)

        kT_sb = persist.tile([128, HKV, NRT, 128], dt.bfloat16)   # [d, kvh, g, k]
        v_sb = persist.tile([128, NRT, HKV, 129], dt.bfloat16)    # [k, g, kvh, d|1]
        krot_sb = persist.tile([128, NRT, HKV * D], dt.bfloat16)  # roped k (pre-norm)
        msk_sb = persist.tile([128, NRT, HKV], dt.float32)
        gate_sb = persist.tile([128, NRT, HKV], dt.float32)

        nc.vector.memset(v_sb[:, :, :, 128:129], 1.0)
        nc.vector.memset(eps_sb, EPS)

        # ---- DMA: consumption order ----
        nc.sync.dma_start(out=wg_sb, in_=wg_d)
        nc.sync.dma_start(out=id_sb, in_=id_d)
        nc.sync.dma_start(out=tri_sb,
                          in_=tri_d.rearrange("p (a n) -> p a n", a=2))
        nc.sync.dma_start(out=npad_sb, in_=npad_d)
        nc.sync.dma_start(out=wkv_sb,
                          in_=wkv_d.rearrange("(a p) n -> p a n", p=128))
        xTv = xT_d.rearrange("(a p) n -> p a n", p=128)
        nc.sync.dma_start(out=xT_sb[:, 0, :], in_=xTv[:, 0, :])  # gate needs full ct0
        csv = cs_d.rearrange("(a p) n -> p a n", p=128)
        vev = ve_d.rearrange("(a p) n -> p a n", p=128)
        nc.sync.dma_start(out=cs_sb[:, 0:8, :], in_=csv[:, 0:8, :])
        nc.sync.dma_start(out=ve_sb[:, 0:8, :], in_=vev[:, 0:8, :])
        for ct in range(1, NCT):  # first halves: rows for rt 0..7
            nc.sync.dma_start(out=xT_sb[:, ct, 0:1024], in_=xTv[:, ct, 0:1024])

        # ---- gate for all rows: 1/(1+exp(-u)) (ve2 is pre-scaled by 2) ----
        with tc.tile_pool(name="gps", bufs=1, space="PSUM") as gps:
            g_psum = gps.tile([128, NRT * HKV], dt.float32)
            for rt in range(NRT):
                nc.tensor.matmul(g_psum[:, bass.ts(rt, HKV)],
                                 lhsT=xT_sb[0:32, 0, bass.ts(rt, 128)],
                                 rhs=wg_sb, start=True, stop=True)
            gflat = gate_sb.rearrange("p a n -> p (a n)")
            nc.scalar.activation(out=gflat, in_=g_psum, func=AF.Exp, scale=-1.0)
            nc.vector.tensor_scalar_add(gflat, gflat, 1.0)
            nc.vector.reciprocal(out=gflat, in_=gflat)

        # late DMAs (needed from rt=8 on)
        nc.sync.dma_start(out=cs_sb[:, 8:16, :], in_=csv[:, 8:16, :])
        nc.sync.dma_start(out=ve_sb[:, 8:16, :], in_=vev[:, 8:16, :])
        for ct in range(NCT):  # second halves: rows for rt 8..15
            nc.sync.dma_start(out=xT_sb[:, ct, 1024:2048], in_=xTv[:, ct, 1024:2048])
        nc.sync.dma_start(out=wq_sb,
                          in_=wq_d.rearrange("(a p) n -> p a n", p=128))
        nc.sync.dma_start(out=wo_sb,
                          in_=wo_d.rearrange("(a p) n -> p a n", p=128))

        # ---- main PSUM pools (8 banks total) ----
        projps = ctx.enter_context(tc.tile_pool(name="projps", bufs=2, space="PSUM"))
        sAps = ctx.enter_context(tc.tile_pool(name="sAps", bufs=1, space="PSUM"))
        sBps = ctx.enter_context(tc.tile_pool(name="sBps", bufs=1, space="PSUM"))
        yps = ctx.enter_context(tc.tile_pool(name="yps", bufs=1, space="PSUM"))
        tpps = ctx.enter_context(tc.tile_pool(name="tpps", bufs=2, space="PSUM"))

        # per-iteration ring state
        qT_cur = [None, None]      # qT tile ring (per qt)
        yN_cur = [None, None]      # yN tile ring (per qt)

        kv_cur = {}

        def kv_mm(rt, lo, hi):
            rs = bass.ts(rt, 128)
            if lo == 0:
                kv_cur[rt] = projps.tile([128, 512], dt.float32, tag="proj",
                                         name="kv")
            kv = kv_cur[rt]
            for ct in range(lo, hi):
                nc.tensor.matmul(kv, lhsT=xT_sb[:, ct, rs], rhs=wkv_sb[:, ct, :],
                                 start=(ct == 0), stop=(ct == NCT - 1))

        def kv_post(rt):
            kv = kv_cur.pop(rt)
            # v = v_raw + gate*ve2  (DVE, reads psum)
            for kvh in range(HKV):
                nc.vector.scalar_tensor_tensor(
                    out=v_sb[:, rt, kvh, 0:128],
                    in0=ve_sb[:, rt, bass.ts(kvh, 128)],
                    scalar=gate_sb[:, rt, kvh:kvh + 1],
                    in1=kv[:, 256 + kvh * 128:256 + (kvh + 1) * 128],
                    op0=ALU.mult, op1=ALU.add)
            kraw = work.tile([128, HKV * D], dt.bfloat16, tag="kraw")
            nc.scalar.copy(out=kraw, in_=kv[:, 0:256])
            return kraw

        def kv_rope(rt, kraw):
            k3 = kraw.rearrange("p (a n) -> p a n", a=HKV)
            kr3 = krot_sb[:, rt, :].rearrange("p (a n) -> p a n", a=HKV)
            ccb = _bcast(cs_sb[:, rt, 0:128], HKV)
            ssb = _bcast(cs_sb[:, rt, 128:256], HKV)
            kc_ = work.tile([128, HKV, 128], dt.bfloat16, tag="t1")
            ks_ = work.tile([128, HKV, 128], dt.bfloat16, tag="t2")
            nc.gpsimd.tensor_mul(kc_, k3, ccb)
            nc.gpsimd.tensor_mul(ks_, k3, ssb)
            nc.gpsimd.tensor_add(kr3, kc_, _halfswap(ks_, HKV))

        def kv_norm(rt):
            # rms stats from the ROPED values (exact for any cos/sin tables);
            # 1/sqrt via exp(-0.5*ln(ms+eps)) to stay in one act table
            kr3 = krot_sb[:, rt, :].rearrange("p (a n) -> p a n", a=HKV)
            ksq = work.tile([128, HKV, D], dt.bfloat16, tag="t1")
            nc.vector.tensor_mul(ksq, kr3, kr3)
            nc.vector.tensor_reduce(out=msk_sb[:, rt, :], in_=ksq,
                                    axis=AX.X, op=ALU.add)
            nc.scalar.activation(out=msk_sb[:, rt, :], in_=msk_sb[:, rt, :],
                                 func=AF.Ln, bias=eps_sb, scale=1.0 / D)
            nc.scalar.activation(out=msk_sb[:, rt, :], in_=msk_sb[:, rt, :],
                                 func=AF.Exp, scale=-0.5)
            for kvh in range(HKV):
                nc.vector.tensor_scalar_mul(
                    kr3[:, kvh, :], kr3[:, kvh, :], msk_sb[:, rt, kvh:kvh + 1])

        def kv_stage(rt):
            kv_mm(rt, 0, NCT)
            kraw = kv_post(rt)
            kv_rope(rt, kraw)
            kv_norm(rt)

        def ktr_stage(rt):
            kr3 = krot_sb[:, rt, :].rearrange("p (a n) -> p a n", a=HKV)
            for kvh in range(HKV):
                ktp = tpps.tile([128, 128], dt.bfloat16, tag="tp")
                nc.tensor.transpose(ktp, kr3[:, kvh, :], id_sb)
                nc.vector.tensor_copy(out=kT_sb[:, kvh, rt, :], in_=ktp)

        def q_mm(qt, half, qraw):
            """Projection matmuls + psum->sbuf copy for heads half*4..half*4+4."""
            rt = qt + NQT
            rs = bass.ts(rt, 128)
            qp = projps.tile([128, 512], dt.float32, tag="proj", name="qp")
            for ct in range(NCT):
                nc.tensor.matmul(qp, lhsT=xT_sb[:, ct, rs],
                                 rhs=wq_sb[:, ct, bass.ts(half, 512)],
                                 start=(ct == 0), stop=(ct == NCT - 1))
            nc.scalar.copy(out=qraw[:, bass.ts(half, 512)], in_=qp)

        def q_rest(qt, half, qraw, qrot, msq):
            """Rope + rms-normalize heads [4*half, 4*half+4) (DVE/Act only)."""
            rt = qt + NQT
            HH = H // 2
            q3 = qraw[:, bass.ts(half, 512)].rearrange("p (a n) -> p a n", a=HH)
            qr3 = qrot[:, bass.ts(half, 512)].rearrange("p (a n) -> p a n", a=HH)
            ccbq = _bcast(cs_sb[:, rt, 0:128], HH)
            ssbq = _bcast(cs_sb[:, rt, 128:256], HH)
            u1 = work.tile([128, HH, 128], dt.bfloat16, tag="u1")
            u2 = work.tile([128, HH, 128], dt.bfloat16, tag="u2")
            nc.vector.tensor_mul(u1, q3, ccbq)
            nc.vector.tensor_mul(u2, q3, ssbq)
            nc.vector.tensor_add(qr3, u1, _halfswap(u2, HH))
            qsq = work.tile([128, HH, D], dt.bfloat16, tag="u1")
            nc.vector.tensor_mul(qsq, qr3, qr3)
            ms = msq[:, bass.ts(half, HH)]
            nc.vector.tensor_reduce(out=ms, in_=qsq, axis=AX.X, op=ALU.add)
            nc.scalar.activation(out=ms, in_=ms, func=AF.Ln,
                                 bias=eps_sb, scale=1.0 / D)
            nc.scalar.activation(out=ms, in_=ms, func=AF.Exp, scale=-0.5)
            for h in range(HH):
                nc.vector.tensor_scalar(
                    out=qr3[:, h, :], in0=qr3[:, h, :],
                    scalar1=ms[:, h:h + 1], scalar2=SCALE,
                    op0=ALU.mult, op1=ALU.mult)

        def qtr_one(qrot, qT, h):
            qr3 = qrot.rearrange("p (a n) -> p a n", a=H)
            qtp = tpps.tile([128, 128], dt.bfloat16, tag="tp")
            nc.tensor.transpose(qtp, qr3[:, h, :], id_sb)
            nc.vector.tensor_copy(out=qT[:, h, :], in_=qtp)

        def d_tr(qt, yT, h):
            """Transpose yN head h of qt into yT (PE + DVE copy)."""
            yN = yN_cur[qt % 2]
            ytp = tpps.tile([128, 128], dt.bfloat16, tag="tp")
            nc.tensor.transpose(ytp, yN[:, h, :], id_sb)
            nc.vector.tensor_copy(out=yT[:, h, :], in_=ytp)

        # output psum tiles for the in-flight D stage (allocated per qt)
        d_state = {}

        def d_open(qt):
            yT = work.tile([128, H, 128], dt.bfloat16, tag="yT")
            d_state[qt] = (yT, [None, None])

        def d_mm(qt, h):
            yT, oo = d_state[qt]
            if h == 0:
                oo[0] = projps.tile([128, 512], dt.float32, tag="proj", name="oo0")
                oo[1] = projps.tile([128, 512], dt.float32, tag="proj", name="oo1")
            for half in range(2):
                nc.tensor.matmul(oo[half], lhsT=yT[:, h, :],
                                 rhs=wo_sb[:, h, bass.ts(half, 512)],
                                 start=(h == 0), stop=(h == H - 1))

        def d_close(qt):
            yT, oo = d_state.pop(qt)
            for half in range(2):
                osb = work.tile([128, 512], dt.float32, tag="osb")
                nc.scalar.copy(out=osb, in_=oo[half])
                nc.sync.dma_start(
                    out=out_d[bass.ts(qt, 128), bass.ts(half, 512)], in_=osb)

        def att_step_scores(qt, h, pA, pB):
            kvh = h // REP
            sA = sAps.tile([128, NCA, 128], dt.float32, tag="sA")
            for kc in range(NCA):
                nc.tensor.matmul(sA[:, kc, :], lhsT=kT_sb[:, kvh, qt + kc, :],
                                 rhs=qT_cur[qt % 2][:, h, :],
                                 start=True, stop=True)
            sB = sBps.tile([128, NCB, 128], dt.float32, tag="sB")
            for kc in range(NCB):
                nc.tensor.matmul(sB[:, kc, :],
                                 lhsT=kT_sb[:, kvh, qt + NCA + kc, :],
                                 rhs=qT_cur[qt % 2][:, h, :],
                                 start=True, stop=True)
            nc.scalar.activation(out=pA.rearrange("p a n -> p (a n)"),
                                 in_=sA.rearrange("p a n -> p (a n)"),
                                 func=AF.Exp)
            nc.scalar.activation(out=pB.rearrange("p a n -> p (a n)"),
                                 in_=sB.rearrange("p a n -> p (a n)"),
                                 func=AF.Exp)
            nc.gpsimd.tensor_mul(pA[:, 0, :], pA[:, 0, :], tri_sb[:, 0, :])
            nc.gpsimd.tensor_mul(pB[:, NCB - 1, :], pB[:, NCB - 1, :],
                                 tri_sb[:, 1, :])

        def att_step_pv(qt, h, pA, pB):
            kvh = h // REP
            y = yps.tile([128, 129], dt.float32, tag="y")
            for kc in range(NKC):
                p = pA[:, kc, :] if kc < NCA else pB[:, kc - NCA, :]
                nc.tensor.matmul(y, lhsT=p, rhs=v_sb[:, qt + kc, kvh, :],
                                 start=(kc == 0), stop=(kc == NKC - 1))
            z = work.tile([128, 1], dt.float32, tag="z")
            nc.vector.tensor_sub(z, y[:, 128:129], npad_sb[:, qt:qt + 1])
            nc.vector.reciprocal(out=z, in_=z)
            nc.vector.tensor_scalar_mul(yN_cur[qt % 2][:, h, :], y[:, 0:128], z)

        # ================= warmup: halo kv (rt 0..7) =================
        for rt in range(NQT):
            kv_stage(rt)
            if rt > 0:
                ktr_stage(rt - 1)

        # ================= main loop (rt 8..15 / qt 0..7) =================
        pwork = ctx.enter_context(tc.tile_pool(name="pwork", bufs=2))
        def new_q_tiles(qt):
            qT_cur[qt % 2] = persist.tile([128, H, 128], dt.bfloat16,
                                          tag=f"qT{qt % 2}", name=f"qT_{qt}")
            yN_cur[qt % 2] = persist.tile([128, H, 128], dt.bfloat16,
                                          tag=f"yN{qt % 2}", name=f"yN_{qt}")
            qraw = work.tile([128, C], dt.bfloat16, tag="qraw")
            qrot = work.tile([128, C], dt.bfloat16, tag="qrot")
            msq = work.tile([128, H], dt.float32, tag="msq")
            return (qraw, qrot, msq)

        # prologue for qt=0: kv(8) + full q-stage(0)
        qst = [None, None]        # (qraw, qrot, msq) ring per qt%2
        kv_stage(NQT)
        qst[0] = new_q_tiles(0)
        q_mm(0, 0, qst[0][0])
        q_rest(0, 0, *qst[0])
        q_mm(0, 1, qst[0][0])
        q_rest(0, 1, *qst[0])
        ktr_stage(NQT - 1)   # ktr(7) from warmup lag
        ktr_stage(NQT)
        qtr_one(qst[0][1], qT_cur[0], 0)

        # steady state: att(qt) is the spine; kv(rt+1), q-half0(qt+1) and the
        # output projection of qt-1 are woven into its steps so PE stays busy
        # while Act paces the exps.
        for qt in range(NQT):
            rt = qt + NQT
            qrot = qst[qt % 2][1]
            pAs = [None, None]
            pBs = [None, None]
            kraw_next = [None]
            for h in range(H):
                pAs[h % 2] = pwork.tile([128, NCA, 128], dt.bfloat16, tag="pA", name="pA")
                pBs[h % 2] = pwork.tile([128, NCB, 128], dt.bfloat16, tag="pB", name="pB")
                if h < H - 1:
                    qtr_one(qrot, qT_cur[qt % 2], h + 1)
                att_step_scores(qt, h, pAs[h % 2], pBs[h % 2])
                if h > 0:
                    att_step_pv(qt, h - 1, pAs[(h - 1) % 2], pBs[(h - 1) % 2])
                if qt > 0:
                    if h < 2:
                        d_tr(qt - 1, d_state[qt - 1][0], 6 + h)
                        d_mm(qt - 1, 6 + h)
                    elif h == 2:
                        d_close(qt - 1)
                if qt < NQT - 1:
                    if h == 4:
                        kv_mm(rt + 1, 0, 4)
                    elif h == 5:
                        kv_mm(rt + 1, 4, NCT)
                        kraw_next[0] = kv_post(rt + 1)
                    elif h == 6:
                        kv_rope(rt + 1, kraw_next[0])
                        qst[(qt + 1) % 2] = new_q_tiles(qt + 1)
                        q_mm(qt + 1, 0, qst[(qt + 1) % 2][0])
                    elif h == 7:
                        kv_norm(rt + 1)
                        q_rest(qt + 1, 0, *qst[(qt + 1) % 2])
            att_step_pv(qt, H - 1, pAs[(H - 1) % 2], pBs[(H - 1) % 2])

            # boundary: q-half1(qt+1), D fillers for qt, ktr/qtr for qt+1
            d_open(qt)
            yT = d_state[qt][0]
            if qt < NQT - 1:
                q_mm(qt + 1, 1, qst[(qt + 1) % 2][0])
                d_tr(qt, yT, 0)
                d_tr(qt, yT, 1)
                for j in range(6):
                    d_mm(qt, j)
                    if j + 2 < 6:
                        d_tr(qt, yT, j + 2)
                q_rest(qt + 1, 1, *qst[(qt + 1) % 2])
                ktr_stage(rt + 1)
                qtr_one(qst[(qt + 1) % 2][1], qT_cur[(qt + 1) % 2], 0)
            else:
                d_tr(qt, yT, 0)
                d_tr(qt, yT, 1)
                for j in range(H):
                    d_mm(qt, j)
                    if j + 2 < H:
                        d_tr(qt, yT, j + 2)
                d_close(qt)


# ---------------------------------------------------------------------------
# host side
# ---------------------------------------------------------------------------

def make_in_maps(x, ve, cos, sin, Wq, Wk, Wv, Wproj, Wg):
    """Build the 8 per-core input dicts (numpy, host-side prep)."""
    x = np.asarray(x, F32)
    ve = np.asarray(ve, F32)
    cos = np.asarray(cos, F32).reshape(T, 64)
    sin = np.asarray(sin, F32).reshape(T, 64)
    Wq = np.asarray(Wq, F32)
    Wk = np.asarray(Wk, F32)
    Wv = np.asarray(Wv, F32)
    Wproj = np.asarray(Wproj, F32)
    Wg = np.asarray(Wg, F32)

    wq = Wq.astype(BF16)
    wkv = np.concatenate([Wk, Wv], axis=1).astype(BF16)
    wo = Wproj.astype(BF16)
    wg = Wg.astype(BF16)
    ident = np.eye(128, dtype=BF16)

    # triangular masks in [k, q] layout
    kk = np.arange(128)[:, None]
    qq = np.arange(128)[None, :]
    tri = np.zeros((128, 2, 128), F32)
    tri[:, 0, :] = np.where(kk < qq, 0.0, 1.0)   # LEFT chunk (kc=0), mult mask
    tri[:, 1, :] = np.where(kk > qq, 0.0, 1.0)   # DIAG chunk (kc=8), mult mask
    tri = tri.reshape(128, 256).astype(BF16)

    in_maps = []
    for c in range(8):
        b, ck = divmod(c, 4)
        t0 = ck * RCHUNK
        es = t0 - WIN  # ext start (may be negative for chunk 0)
        pad = max(0, -es)

        def ext(a, fill_shape):
            out = np.zeros((E,) + fill_shape, F32)
            out[pad:] = a[es + pad: t0 + RCHUNK]
            return out

        x_e = ext(x[b], (C,))
        ve_e = ext(ve[b], (HKV * D,))
        cos_e = ext(cos, (64,))
        sin_e = ext(sin, (64,))

        npad = np.zeros((128, NQT), F32)
        if pad:
            kc = np.arange(NKC)[:, None]
            kl = np.arange(128)[None, :]
            r = np.arange(128)
            for qt in range(NQT):
                extpos = 128 * (qt + kc) + kl          # [9, 128]
                is_pad = extpos < pad
                for ri in r:
                    tri_ok = np.ones((NKC, 128), bool)
                    tri_ok[0] = kl[0] >= ri
                    tri_ok[NKC - 1] = kl[0] <= ri
                    npad[ri, qt] = np.sum(tri_ok & is_pad)

        in_maps.append({
            "xT": np.ascontiguousarray(x_e.T).astype(BF16),
            "wq": wq, "wkv": wkv, "wo": wo, "wg": wg,
            "ve2": (2.0 * ve_e).astype(BF16),
            "cs": np.concatenate([cos_e, cos_e, -sin_e, sin_e],
                                 axis=1).astype(BF16),
            "tri": tri, "npad": npad, "ident": ident,
        })
    return in_maps


_NC_CACHE = None


def kernel(x, ve, cos, sin, Wq, Wk, Wv, Wproj, Wg, window_size):
    assert int(window_size) == WIN
    global _NC_CACHE
    if _NC_CACHE is None:
        _NC_CACHE = build_nc()
    nc = _NC_CACHE
    in_maps = make_in_maps(x, ve, cos, sin, Wq, Wk, Wv, Wproj, Wg)
    res = bass_utils.run_bass_kernel_spmd(nc, in_maps, core_ids=list(range(8)))
    out = np.zeros((B, T, C), F32)
    for c in range(8):
        b, ck = divmod(c, 4)
        out[b, ck * RCHUNK:(ck + 1) * RCHUNK] = res.results[c]["out"]
    return out
